# revision 22
# baseline (speedup 1.0000x reference)
"""Trainium2 Bass kernel for DiscreteDeltaThetaGammaLayer.

Coupled Kuramoto-oscillator recurrence:
  phase0 = (x @ W_phase.T) mod 2pi ; amp0 = max(|x @ W_amp.T|, eps)
  32 steps of: intra-band Kuramoto coupling (phase), PAC amplitude modulation
  output: final amp  (4096, 352) f32

Strategy (8 NeuronCores, data-parallel over batch, 512 rows/core):
  - State transposed [128 osc partitions x batch free]; oscillators permuted
    into chunks: c0 = delta(32)+theta(64)+pad(32), c1/c2 = gamma halves.
  - Rotating frame per band: phi~ = phi - k*dt*omega_band. The per-step
    dt*omega add AND the wrap disappear (coupling drift <= 0.02 rad/step,
    32 steps => |phi~| <= pi+0.65 where the Sin LUT still has ~1e-3 abs err).
    Host de-rotates the stashed band sums exactly in f64.
  - bf16 state + coupling matmuls (full PE rate at any width), f32r amp path.
  - Band sums (Sd St Cd Ct) fused into the chunk-0 coupling matmul: the
    K-block's 32 pad lhsT columns carry delta/theta indicator columns, so
    PSUM partitions 96:98 of vu hold the band sums for free.
  - Per step per stream: 2 ACT sin (cos via sin(pi/2-|phi|)), 1 ACT abs,
    10 PE matmuls, DVE: mm=cs*vu, d=mm1-mm2, phi+=d (bf16 TT at 2x),
    Pool: stash copy. Host reconstructs amp exactly (clamped-recurrence
    closed form) from stashed band sums.
  - Fallback (general omega / huge coupling): no rotating frame, per-step
    custom-DVE wrap with dt*omega folded in (s0).
"""

import math
import sys

sys.path.insert(0, "/opt/trn_rl_repo")

import numpy as np

# ---- problem constants (module hyperparameters) ----
N_DELTA, N_THETA, N_GAMMA = 32, 64, 256
N_TOTAL = 352
N_DIMS = 1024
BATCH = 4096
N_STEPS = 32
DT = 0.01
PAC = 0.3
EPS = 1e-6
TWO_PI = 2.0 * math.pi
PI = math.pi

N_CORES = 8
BL = BATCH // N_CORES          # 512 batch rows per core
BHS = [256, 256]               # independent streams (latency hiding)
OFFS = [0, 256]
NH = len(BHS)
P = 128
NCH = 3                        # oscillator chunks (3*128 = 384 >= 352)
CHUNK_REAL = [96, 128, 128]
KD = N_DIMS // P               # 8 contraction chunks for the projections

LAST_EXEC_NS = None
_COMPILED = {}
_WRAP_SUB = None

# drift budget: |phi~| may reach pi + DRIFT_MAX with Sin LUT err ~1.2e-3
DRIFT_MAX = 0.66
STRIDE = 32                    # coupling super-step (reference steps per iter)


def _osc_perm():
    """orig oscillator index for each (chunk, partition); -1 for pads."""
    perm = -np.ones((NCH, P), dtype=np.int64)
    perm[0, :96] = np.arange(96)           # delta + theta
    perm[1, :] = 96 + np.arange(128)       # gamma 0:128
    perm[2, :] = 224 + np.arange(128)      # gamma 128:256
    return perm


def _get_wrap_sub():
    """Custom DVE op: out = wrap((in0 - in1) + s0) into [-s1, s1], period imm2."""
    global _WRAP_SUB
    if _WRAP_SUB is not None:
        return _WRAP_SUB
    from concourse.dve_spec import C0, C1, C2, Spec, Src0, Src1, lower
    from concourse.dve_uop import DveOpSpec
    import concourse.dve_ops as dvo

    def _ref(in0, in1, s0, s1, imm2):
        y = (in0 - in1) + s0
        return (y + imm2 * ((y < -s1).astype(np.float32)
                            - (y > s1).astype(np.float32))).astype(np.float32)

    _y = (Src0 - Src1) + C0
    spec = Spec(body=_y + C2 * ((_y < -C1) - (_y > C1)), reference=_ref)
    shas = {}
    for ver in ("v3", "v4"):
        tmp = DveOpSpec(name="WRAP_SUB_KERNEL", opcode=31,
                        uops=lower(spec, ver=ver), rd1_en=True)
        shas[ver] = tmp.sha(ver)
    op = dvo.DveOp("WRAP_SUB_KERNEL", spec, subdim=False, uops_sha=shas)
    dvo.OPS.append(op)
    dvo.CUSTOM_DVE_SPECS[op.name] = op.spec
    dvo._SUB_OPCODE_FOR_NAME[op.name] = dvo._CUSTOM_DVE_ROW_BASE + len(dvo.OPS) - 1
    _WRAP_SUB = op
    return op


def _build_program(nz_pairs, fast_rot, has_res):
    """fast_rot: rotating frame, no wrap, stride-2 coupling (17 iterations).
    Fallback: per-step wrap, stride-1 (33 iterations)."""
    import concourse.bass as bass
    import concourse.tile as tile
    from concourse import bacc, mybir

    f32 = mybir.dt.float32
    f32r = mybir.dt.float32r
    bf16 = mybir.dt.bfloat16
    AF = mybir.ActivationFunctionType
    ALU = mybir.AluOpType

    wrap_sub = _get_wrap_sub() if not fast_rot else None

    nc = bacc.Bacc("TRN2", target_bir_lowering=False, debug=False)

    # ---- DRAM I/O (host pre-packs k-chunks along the free dim) ----
    xbT = nc.dram_tensor("xbT", [P, KD * BL], bf16, kind="ExternalInput").ap()
    wpT = nc.dram_tensor("wpT", [P, KD * NCH * P], bf16,
                         kind="ExternalInput").ap()
    xfT = nc.dram_tensor("xfT", [P, KD * BL], f32r, kind="ExternalInput").ap()
    waT = nc.dram_tensor("waT", [P, KD * NCH * P], f32r,
                         kind="ExternalInput").ap()
    ktT = nc.dram_tensor("ktT", [P, NCH * NCH * P], bf16,
                         kind="ExternalInput").ap()
    # per-(partition,chunk) scalars: residual r (fast path) or dt*omega (fallback)
    dtw = nc.dram_tensor("dtw", [P, NCH], f32, kind="ExternalInput").ap()
    # tap-partition phase init rows: [0, 0, pi/2, pi/2] x bh
    padphi = nc.dram_tensor("padphi", [4, max(BHS)], bf16,
                            kind="ExternalInput").ap()

    amp0_out = nc.dram_tensor("amp0", [P, NCH * BL], f32,
                              kind="ExternalOutput").ap()
    aoff = [NCH * sum(BHS[:h]) for h in range(NH)]
    # iterations: stride-2 (fast) dumps sums of phi_{2m}, m=0..MS-1;
    # stride-1 (fallback) dumps sums of phi_it, it=1..N_STEPS
    MS = (N_STEPS // STRIDE + 1) if fast_rot else (N_STEPS + 1)
    NDUMP = MS if fast_rot else N_STEPS
    # stash: rows (Sd, St, -Cd, -Ct); per stream block of NDUMP*bh cols
    bs_out = nc.dram_tensor("bsums", [4, NDUMP * BL], bf16,
                            kind="ExternalOutput").ap()

    with tile.TileContext(nc) as tc:
        with (
            tc.tile_pool(name="state", bufs=1) as state_pool,
            tc.tile_pool(name="weights", bufs=1) as wpool,
            tc.tile_pool(name="work", bufs=2) as work,
            tc.tile_pool(name="psum", bufs=1, space="PSUM") as psum,
        ):
            # ---- persistent constants ----
            dtw_sb = None
            if (not fast_rot) or has_res:
                dtw_sb = wpool.tile([P, NCH], f32, tag="dtw", name="dtw_sb")
                nc.scalar.dma_start(dtw_sb[:], dtw[:])
            pihalf = wpool.tile([P, 1], f32, tag="pihalf", name="pihalf")
            nc.vector.memset(pihalf[:], PI / 2.0)
            # touch Sin once so the ACT table loads during the input DMAs
            warm = wpool.tile([P, 1], bf16, tag="warm", name="warm")
            nc.scalar.activation(warm[:], pihalf[:], AF.Sin)

            ktall = wpool.tile([P, NCH * NCH * P], bf16, tag="ktall",
                               name="ktall")
            nc.scalar.dma_start(ktall[:], ktT[:])
            kt_sb = {}
            for (jc, ic) in nz_pairs:
                o = (jc * NCH + ic) * P
                kt_sb[(jc, ic)] = ktall[:, o:o + P]

            # ---- big input loads (phase path first: it gates the recurrence)
            xall = wpool.tile([P, KD * BL], bf16, tag="xall", name="xall")
            wall = wpool.tile([P, KD * NCH * P], bf16, tag="wall", name="wall")
            half = KD // 2
            nc.sync.dma_start(xall[:, 0:half * BL], xbT[:, 0:half * BL])
            nc.scalar.dma_start(wall[:, 0:half * NCH * P],
                                wpT[:, 0:half * NCH * P])
            nc.sync.dma_start(xall[:, half * BL:], xbT[:, half * BL:])
            nc.scalar.dma_start(wall[:, half * NCH * P:],
                                wpT[:, half * NCH * P:])
            xk = [xall[:, k * BL:(k + 1) * BL] for k in range(KD)]
            wk = [wall[:, k * NCH * P:(k + 1) * NCH * P] for k in range(KD)]
            xfall = wpool.tile([P, KD * BL], f32r, tag="xfall", name="xfall")
            waall = wpool.tile([P, KD * NCH * P], f32r, tag="waall",
                               name="waall")
            nc.gpsimd.dma_start(xfall[:], xfT[:])
            nc.gpsimd.dma_start(waall[:], waT[:])
            xfk = [xfall[:, k * BL:(k + 1) * BL] for k in range(KD)]
            wak = [waall[:, k * NCH * P:(k + 1) * NCH * P] for k in range(KD)]

            boff = [NDUMP * sum(BHS[:h]) for h in range(NH)]
            # ---- per-stream state ----
            phi, cs, mmt, dts, vu = [], [], [], [], []
            ucop = []
            for h in range(NH):
                bh = BHS[h]
                wh = NCH * bh
                phi.append(state_pool.tile([P, wh], bf16, tag=f"phi{h}",
                                           name=f"phi{h}"))
                ucop.append(state_pool.tile([P, wh], bf16, tag=f"ucop{h}",
                                            name=f"ucop{h}"))
                cs.append(state_pool.tile([P, 2 * wh], bf16, tag=f"cs{h}",
                                          name=f"cs{h}"))
                mmt.append(state_pool.tile([P, 2 * wh], bf16, tag=f"mm{h}",
                                           name=f"mm{h}"))
                dts.append([state_pool.tile([P, wh], bf16, tag=f"d{h}_{pb}",
                                            name=f"d{h}_{pb}")
                            for pb in range(2)])
                vu.append(psum.tile([P, 2 * wh], f32, tag=f"vu{h}",
                                    name=f"vu{h}"))
            amp_acc = psum.tile([P, NCH * max(BHS)], f32, tag="ampacc",
                                name="amp_acc")
            pabs = [work.tile([P, NCH * BHS[h]], bf16, tag=f"pabs{h}",
                              name=f"pabs{h}") for h in range(NH)]

            # ---- phase projections -> phi (per stream) ----
            for h in range(NH):
                bh = BHS[h]
                wh = NCH * bh
                for c in range(NCH):
                    acc = vu[h][:, c * bh:(c + 1) * bh]
                    for k in range(KD):
                        nc.tensor.matmul(
                            acc, wk[k][:, c * P:(c + 1) * P],
                            xk[k][:, OFFS[h]:OFFS[h] + bh],
                            start=(k == 0), stop=(k == KD - 1),
                        )
                nc.vector.add_range_wrap(phi[h][:], vu[h][:, 0:wh],
                                         0.0, PI, TWO_PI)
                # pad partitions of chunk 0 carry band-sum taps:
                # 96,97 keep phi=0 (cos=1,sin=0); 98,99 get pi/2 (cos=0,sin=1)
                nc.scalar.dma_start(phi[h][96:100, 0:bh], padphi[:, 0:bh])

            # ---- the recurrence ----
            def emit_amp_path():
                for h in range(NH):
                    bh = BHS[h]
                    wh = NCH * bh
                    for c in range(NCH):
                        acc = amp_acc[:, c * bh:(c + 1) * bh]
                        for k in range(KD):
                            nc.tensor.matmul(
                                acc, wak[k][:, c * P:(c + 1) * P],
                                xfk[k][:, OFFS[h]:OFFS[h] + bh],
                                start=(k == 0), stop=(k == KD - 1),
                            )
                    ab = work.tile([P, wh], f32, tag=f"abs0_{h}",
                                   name=f"abs0_{h}")
                    nc.scalar.activation(ab[:], amp_acc[:, 0:wh], AF.Abs)
                    nc.sync.dma_start(
                        amp0_out[:, aoff[h]:aoff[h] + wh], ab[:])

            amp_at = min(2, MS - 1)
            for it in range(MS):
                if it == amp_at:
                    emit_amp_path()
                for h in range(NH):
                    bh = BHS[h]
                    wh = NCH * bh
                    ph = phi[h]
                    sin = cs[h][:, wh:2 * wh]
                    cos = cs[h][:, 0:wh]
                    # last iteration only feeds the chunk-0 band sums
                    last = (it == MS - 1)
                    cw = bh if last else wh
                    nc.scalar.activation(sin[:, 0:cw], ph[:, 0:cw], AF.Sin)
                    u16 = mybir.dt.uint16
                    nc.vector.tensor_scalar(
                        pabs[h][:, 0:cw].bitcast(u16),
                        ph[:, 0:cw].bitcast(u16),
                        0x7FFF, None, ALU.bitwise_and)
                    nc.scalar.activation(cos[:, 0:cw], pabs[h][:, 0:cw], AF.Sin,
                                         bias=pihalf[:], scale=-1.0)

                    # coupling: [v | u] = (dt*K) [sin | cos]; chunk-0 block
                    # also emits band sums on partitions 96:98
                    for ic in range(NCH):
                        if last and ic > 0:
                            continue
                        jcs = [jc for (jc, i2) in nz_pairs if i2 == ic]
                        for half, srcoff in ((0, wh), (1, 0)):
                            dst = vu[h][:, half * wh + ic * bh:
                                        half * wh + (ic + 1) * bh]
                            for n, jc in enumerate(jcs):
                                src = cs[h][:, srcoff + jc * bh:
                                            srcoff + (jc + 1) * bh]
                                nc.tensor.matmul(
                                    dst, kt_sb[(jc, ic)], src,
                                    start=(n == 0), stop=(n == len(jcs) - 1),
                                )

                    # stash band sums of post-update phase (it >= 1):
                    # vu partitions 96:98, chunk0 of each half -> stash cols
                    if last:
                        # band sums only: mm and d on tap partitions, chunk 0
                        for half in (0, 1):
                            nc.vector.tensor_tensor(
                                mmt[h][96:128, half * wh:half * wh + bh],
                                cs[h][96:128, half * wh:half * wh + bh],
                                vu[h][96:128, half * wh:half * wh + bh],
                                ALU.mult)
                        a, b = (0, wh) if fast_rot else (wh, 0)
                        dtile = dts[h][it % 2]
                        nc.vector.tensor_tensor(
                            dtile[96:100, 0:bh],
                            mmt[h][96:100, a:a + bh],
                            mmt[h][96:100, b:b + bh], ALU.subtract)
                        slot = it if fast_rot else it - 1
                        so = boff[h] + slot * bh
                        nc.sync.dma_start(bs_out[:, so:so + bh],
                                          dtile[96:100, 0:bh])
                        continue

                    # mm = [cos|sin] * [v|u]
                    nc.vector.tensor_tensor(mmt[h][:], cs[h][:], vu[h][:],
                                            ALU.mult)
                    # fast path: d = c*v - s*u (= coup); fallback: d = -coup
                    # since WRAP_SUB computes wrap((phi - d) + s0).
                    # tap partitions 96:100 of chunk 0 hold (Sd, St, -Cd, -Ct)
                    # (negated in fallback mode).
                    dtile = dts[h][it % 2]
                    a, b = (0, wh) if fast_rot else (wh, 0)
                    nc.vector.tensor_tensor(
                        dtile[:], mmt[h][:, a:a + wh],
                        mmt[h][:, b:b + wh], ALU.subtract)
                    if fast_rot or it > 0:
                        slot = it if fast_rot else it - 1
                        so = boff[h] + slot * bh
                        nc.sync.dma_start(bs_out[:, so:so + bh],
                                          dtile[96:100, 0:bh])
                    if fast_rot:
                        if has_res:
                            for c in range(NCH):
                                pe = 96 if c == 0 else P
                                nc.vector.scalar_tensor_tensor(
                                    ph[0:pe, c * bh:(c + 1) * bh],
                                    dtile[0:pe, c * bh:(c + 1) * bh],
                                    dtw_sb[0:pe, c:c + 1],
                                    ph[0:pe, c * bh:(c + 1) * bh],
                                    ALU.add, ALU.add)
                        else:
                            nc.vector.tensor_tensor(
                                ph[0:96, 0:bh], ph[0:96, 0:bh],
                                dtile[0:96, 0:bh], ALU.add)
                            nc.vector.tensor_tensor(
                                ph[:, bh:wh], ph[:, bh:wh],
                                dtile[:, bh:wh], ALU.add)
                    else:
                        for c in range(NCH):
                            pe = 96 if c == 0 else P
                            nc.vector._custom_dve(
                                wrap_sub,
                                out=ph[0:pe, c * bh:(c + 1) * bh],
                                in0=ph[0:pe, c * bh:(c + 1) * bh],
                                in1=dtile[0:pe, c * bh:(c + 1) * bh],
                                s0=dtw_sb[0:pe, c:c + 1],
                                s1=PI,
                                imm2=TWO_PI,
                            )



    nc.compile()
    return nc


def kernel(x, W_phase, W_amp, omega, K):
    import ml_dtypes
    from concourse.bass_utils import run_bass_kernel_spmd

    x = np.asarray(x, dtype=np.float32)
    W_phase = np.asarray(W_phase, dtype=np.float32)
    W_amp = np.asarray(W_amp, dtype=np.float32)
    omega = np.asarray(omega, dtype=np.float32)
    K = np.asarray(K, dtype=np.float32)

    perm = _osc_perm()
    band_of = np.zeros(N_TOTAL, dtype=np.int64)
    band_of[N_DELTA:N_DELTA + N_THETA] = 1
    band_of[N_DELTA + N_THETA:] = 2

    # ---- rotating-frame feasibility ----
    dtww = DT * omega.astype(np.float64)
    A_band = np.array([dtww[band_of == b].mean() for b in range(3)])
    res = dtww - A_band[band_of]                      # per-osc residual
    # coupling drift bound
    row_l1 = DT * np.abs(K.astype(np.float64)).sum(axis=1)
    drift = N_STEPS * (np.abs(res) + row_l1).max()
    # coupled pairs must share a frame rate
    ii, jj = np.nonzero(K)
    frames_ok = np.allclose(A_band[band_of[ii]], A_band[band_of[jj]],
                            rtol=0, atol=1e-12) if len(ii) else True
    fast_rot = bool(frames_ok and drift <= DRIFT_MAX)
    has_res = bool(fast_rot and np.abs(res).max() > 1e-12)

    # ---- host-side packing ----
    wpT = np.zeros((N_DIMS, NCH * P), dtype=ml_dtypes.bfloat16)
    waT = np.zeros((N_DIMS, NCH * P), dtype=np.float32)

    def chunk_pack(a):
        # [N_DIMS, C] -> [128, KD*C] with k-chunks along free dim
        C = a.shape[1]
        return np.ascontiguousarray(
            a.reshape(KD, P, C).transpose(1, 0, 2).reshape(P, KD * C))
    dtw = np.zeros((P, NCH), dtype=np.float32)
    for c in range(NCH):
        n = CHUNK_REAL[c]
        idx = perm[c, :n]
        wpT[:, c * P:c * P + n] = W_phase[idx].T.astype(ml_dtypes.bfloat16)
        waT[:, c * P:c * P + n] = W_amp[idx].T
        if fast_rot:
            dtw[:n, c] = float(STRIDE) * res[idx].astype(np.float32)
        else:
            w = dtww[idx]
            dtw[:n, c] = (np.mod(w + PI, TWO_PI) - PI).astype(np.float32)

    kT = np.zeros((NCH * P, NCH * P), dtype=np.float32)
    for jc in range(NCH):
        nj = CHUNK_REAL[jc]
        jdx = perm[jc, :nj]
        for ic in range(NCH):
            ni = CHUNK_REAL[ic]
            idx = perm[ic, :ni]
            kT[jc * P:jc * P + nj, ic * P:ic * P + ni] = \
                (float(STRIDE) if fast_rot else 1.0) * DT * K[np.ix_(idx, jdx)].T

    nz = [
        (jc, ic)
        for jc in range(NCH)
        for ic in range(NCH)
        if np.any(kT[jc * P:(jc + 1) * P, ic * P:(ic + 1) * P] != 0.0)
    ]
    if (0, 0) not in nz:
        nz.append((0, 0))     # carries the band-sum indicator columns
    for ic in range(1, NCH):
        if not any(i2 == ic for (_, i2) in nz):
            nz.append((ic, ic))
    nz_pairs = tuple(sorted(nz))

    # fuse delta/theta indicator columns into the (0,0) block pads:
    # cols 96,97 tap the sin half (phi_pad=0), cols 98,99 the cos half
    # (phi_pad=pi/2)
    for cc in (96, 98):
        kT[0:N_DELTA, cc] = 1.0
        kT[N_DELTA:96, cc + 1] = 1.0
    ktT = kT.astype(ml_dtypes.bfloat16)

    key = (nz_pairs, fast_rot, has_res)
    if key not in _COMPILED:
        _COMPILED[key] = _build_program(nz_pairs, fast_rot, has_res)
    nc = _COMPILED[key]

    # kt blocks packed [128, (jc*NCH+ic)*128 .. +128]
    ktp = np.zeros((P, NCH * NCH * P), dtype=ml_dtypes.bfloat16)
    for jc in range(NCH):
        for ic in range(NCH):
            ktp[:, (jc * NCH + ic) * P:(jc * NCH + ic + 1) * P] =                 ktT[jc * P:(jc + 1) * P, ic * P:(ic + 1) * P]
    wpp = chunk_pack(wpT.astype(np.float32)).astype(ml_dtypes.bfloat16)
    wap = chunk_pack(waT)
    padphi = np.zeros((4, max(BHS)), dtype=ml_dtypes.bfloat16)
    padphi[2:4, :] = np.float32(PI / 2.0)
    in_maps = []
    for i in range(N_CORES):
        xs = x[i * BL:(i + 1) * BL]
        xst = np.ascontiguousarray(xs.T)
        xsp = chunk_pack(xst)
        in_maps.append({
            "xbT": xsp.astype(ml_dtypes.bfloat16), "xfT": xsp,
            "wpT": wpp, "waT": wap, "ktT": ktp, "dtw": dtw,
            "padphi": padphi,
        })

    res_run = run_bass_kernel_spmd(nc, in_maps, core_ids=list(range(N_CORES)))

    # ---- host-side unshard + exact amp reconstruction (f64) ----
    out = np.empty((BATCH, N_TOTAL), dtype=np.float32)
    ks = np.arange(1, N_STEPS + 1, dtype=np.float64)   # stash it index
    # de-rotation phases per band (delta for theta-mod, theta for gamma-mod)
    if fast_rot:
        rotd = ks * A_band[0]
        rott = ks * A_band[1]
    else:
        rotd = np.zeros(N_STEPS)
        rott = np.zeros(N_STEPS)

    NDUMP = (N_STEPS // STRIDE + 1) if fast_rot else N_STEPS
    # map true step k=1..32 to dump index (fast: dump m = sums of phi_{S*m})
    if fast_rot:
        kk = np.arange(1, N_STEPS + 1)
        dmap = kk // STRIDE
    else:
        dmap = np.arange(N_STEPS)
    for i in range(N_CORES):
        r = res_run.results[i]
        amp0v = np.maximum(np.abs(r["amp0"].astype(np.float64)), EPS)
        bsv = r["bsums"].astype(np.float64)      # [4, NDUMP*BL]
        if not fast_rot:
            bsv = -bsv                           # fallback d = -coup sign
        # per-stream decode -> f-factors [BL, N_STEPS, {theta, gamma}]
        f = np.empty((BL, N_STEPS, 2))
        off = 0
        for h in range(NH):
            bh = BHS[h]
            blk = bsv[:, off:off + NDUMP * bh].reshape(4, NDUMP, bh)
            blk = blk[:, dmap]                    # expand to N_STEPS
            S = blk[0:2]                          # [2(d,t), k, j] sin sums
            C = -blk[2:4]
            R = np.sqrt(S * S + C * C)
            R = np.maximum(R, 1e-30)
            # true cos(mean phase) = (C cos(kA) - S sin(kA)) / R
            cd = (C[0] * np.cos(rotd)[:, None]
                  - S[0] * np.sin(rotd)[:, None]) / R[0]
            ct = (C[1] * np.cos(rott)[:, None]
                  - S[1] * np.sin(rott)[:, None]) / R[1]
            sl = slice(OFFS[h], OFFS[h] + bh)
            f[sl, :, 0] = 1.0 + DT * PAC * cd.T   # theta-band factor
            f[sl, :, 1] = 1.0 + DT * PAC * ct.T   # gamma-band factor
            off += NDUMP * bh
        Pk = np.cumprod(f, axis=1)                # [BL, k, 2]
        m = np.minimum.accumulate(Pk, axis=1)
        Pn = Pk[:, -1]                            # [BL, 2]
        mn = m[:, -1]
        Pfac = np.ones((BL, 3))
        Efac = np.ones((BL, 3))
        Pfac[:, 1:] = Pn
        Efac[:, 1:] = Pn / mn
        a0 = np.empty((BL, N_TOTAL))
        ao = 0
        for h in range(NH):
            bh = BHS[h]
            for c in range(NCH):
                n = CHUNK_REAL[c]
                idx = perm[c, :n]
                a0[OFFS[h]:OFFS[h] + bh, idx] =                     amp0v[:n, ao + c * bh:ao + (c + 1) * bh].T
            ao += NCH * bh
        amp = np.maximum(a0 * Pfac[:, band_of], EPS * Efac[:, band_of])
        out[i * BL:(i + 1) * BL] = amp.astype(np.float32)
    return out


# revision 23
# speedup vs baseline: 1.3740x; 1.3740x over previous
"""Trainium2 Bass kernel for DiscreteDeltaThetaGammaLayer.

Coupled Kuramoto-oscillator recurrence:
  phase0 = (x @ W_phase.T) mod 2pi ; amp0 = max(|x @ W_amp.T|, eps)
  32 steps of: intra-band Kuramoto coupling (phase), PAC amplitude modulation
  output: final amp  (4096, 352) f32

Strategy (8 NeuronCores, data-parallel over batch, 512 rows/core):
  - State transposed [128 osc partitions x batch free]; oscillators permuted
    into chunks: c0 = delta(32)+theta(64)+pad(32), c1/c2 = gamma halves.
  - Rotating frame per band: phi~ = phi - k*dt*omega_band. The per-step
    dt*omega add AND the wrap disappear (coupling drift <= 0.02 rad/step,
    32 steps => |phi~| <= pi+0.65 where the Sin LUT still has ~1e-3 abs err).
    Host de-rotates the stashed band sums exactly in f64.
  - bf16 state + coupling matmuls (full PE rate at any width), f32r amp path.
  - Band sums (Sd St Cd Ct) fused into the chunk-0 coupling matmul: the
    K-block's 32 pad lhsT columns carry delta/theta indicator columns, so
    PSUM partitions 96:98 of vu hold the band sums for free.
  - Per step per stream: 2 ACT sin (cos via sin(pi/2-|phi|)), 1 ACT abs,
    10 PE matmuls, DVE: mm=cs*vu, d=mm1-mm2, phi+=d (bf16 TT at 2x),
    Pool: stash copy. Host reconstructs amp exactly (clamped-recurrence
    closed form) from stashed band sums.
  - Fallback (general omega / huge coupling): no rotating frame, per-step
    custom-DVE wrap with dt*omega folded in (s0).
"""

import math
import sys

sys.path.insert(0, "/opt/trn_rl_repo")

import numpy as np

# ---- problem constants (module hyperparameters) ----
N_DELTA, N_THETA, N_GAMMA = 32, 64, 256
N_TOTAL = 352
N_DIMS = 1024
BATCH = 4096
N_STEPS = 32
DT = 0.01
PAC = 0.3
EPS = 1e-6
TWO_PI = 2.0 * math.pi
PI = math.pi

N_CORES = 8
BL = BATCH // N_CORES          # 512 batch rows per core
BHS = [256, 256]               # independent streams (latency hiding)
OFFS = [0, 256]
NH = len(BHS)
P = 128
NCH = 3                        # oscillator chunks (3*128 = 384 >= 352)
CHUNK_REAL = [96, 128, 128]
KD = N_DIMS // P               # 8 contraction chunks for the projections

LAST_EXEC_NS = None
_COMPILED = {}
_WRAP_SUB = None

# drift budget: |phi~| may reach pi + DRIFT_MAX with Sin LUT err ~1.2e-3
DRIFT_MAX = 0.66
STRIDE = 32                    # coupling super-step (reference steps per iter)


def _osc_perm():
    """orig oscillator index for each (chunk, partition); -1 for pads."""
    perm = -np.ones((NCH, P), dtype=np.int64)
    perm[0, :96] = np.arange(96)           # delta + theta
    perm[1, :] = 96 + np.arange(128)       # gamma 0:128
    perm[2, :] = 224 + np.arange(128)      # gamma 128:256
    return perm


def _get_wrap_sub():
    """Custom DVE op: out = wrap((in0 - in1) + s0) into [-s1, s1], period imm2."""
    global _WRAP_SUB
    if _WRAP_SUB is not None:
        return _WRAP_SUB
    from concourse.dve_spec import C0, C1, C2, Spec, Src0, Src1, lower
    from concourse.dve_uop import DveOpSpec
    import concourse.dve_ops as dvo

    def _ref(in0, in1, s0, s1, imm2):
        y = (in0 - in1) + s0
        return (y + imm2 * ((y < -s1).astype(np.float32)
                            - (y > s1).astype(np.float32))).astype(np.float32)

    _y = (Src0 - Src1) + C0
    spec = Spec(body=_y + C2 * ((_y < -C1) - (_y > C1)), reference=_ref)
    shas = {}
    for ver in ("v3", "v4"):
        tmp = DveOpSpec(name="WRAP_SUB_KERNEL", opcode=31,
                        uops=lower(spec, ver=ver), rd1_en=True)
        shas[ver] = tmp.sha(ver)
    op = dvo.DveOp("WRAP_SUB_KERNEL", spec, subdim=False, uops_sha=shas)
    dvo.OPS.append(op)
    dvo.CUSTOM_DVE_SPECS[op.name] = op.spec
    dvo._SUB_OPCODE_FOR_NAME[op.name] = dvo._CUSTOM_DVE_ROW_BASE + len(dvo.OPS) - 1
    _WRAP_SUB = op
    return op


def _build_program(nz_pairs, fast_rot, has_res):
    """fast_rot: rotating frame, no wrap, stride-2 coupling (17 iterations).
    Fallback: per-step wrap, stride-1 (33 iterations)."""
    import concourse.bass as bass
    import concourse.tile as tile
    from concourse import bacc, mybir

    f32 = mybir.dt.float32
    f32r = mybir.dt.float32r
    bf16 = mybir.dt.bfloat16
    AF = mybir.ActivationFunctionType
    ALU = mybir.AluOpType

    wrap_sub = _get_wrap_sub() if not fast_rot else None

    nc = bacc.Bacc("TRN2", target_bir_lowering=False, debug=False)

    # ---- DRAM I/O (host pre-packs k-chunks along the free dim) ----
    xbT = nc.dram_tensor("xbT", [P, KD * BL], bf16, kind="ExternalInput").ap()
    wpT = nc.dram_tensor("wpT", [P, KD * NCH * P], bf16,
                         kind="ExternalInput").ap()
    waT = nc.dram_tensor("waT", [P, KD * NCH * P], bf16,
                         kind="ExternalInput").ap()
    ktT = nc.dram_tensor("ktT", [P, NCH * NCH * P], bf16,
                         kind="ExternalInput").ap()
    # per-(partition,chunk) scalars: residual r (fast path) or dt*omega (fallback)
    dtw = nc.dram_tensor("dtw", [P, NCH], f32, kind="ExternalInput").ap()
    # tap-partition phase init rows: [0, 0, pi/2, pi/2] x bh
    padphi = nc.dram_tensor("padphi", [4, max(BHS)], bf16,
                            kind="ExternalInput").ap()

    amp0_out = nc.dram_tensor("amp0", [P, NCH * BL], f32,
                              kind="ExternalOutput").ap()
    aoff = [NCH * sum(BHS[:h]) for h in range(NH)]
    # iterations: stride-2 (fast) dumps sums of phi_{2m}, m=0..MS-1;
    # stride-1 (fallback) dumps sums of phi_it, it=1..N_STEPS
    MS = (N_STEPS // STRIDE + 1) if fast_rot else (N_STEPS + 1)
    NDUMP = MS if fast_rot else N_STEPS
    # stash: rows (Sd, St, -Cd, -Ct); per stream block of NDUMP*bh cols
    bs_out = nc.dram_tensor("bsums", [4, NDUMP * BL], bf16,
                            kind="ExternalOutput").ap()

    with tile.TileContext(nc) as tc:
        with (
            tc.tile_pool(name="state", bufs=1) as state_pool,
            tc.tile_pool(name="weights", bufs=1) as wpool,
            tc.tile_pool(name="work", bufs=2) as work,
            tc.tile_pool(name="psum", bufs=1, space="PSUM") as psum,
        ):
            # ---- persistent constants ----
            dtw_sb = None
            if (not fast_rot) or has_res:
                dtw_sb = wpool.tile([P, NCH], f32, tag="dtw", name="dtw_sb")
                nc.scalar.dma_start(dtw_sb[:], dtw[:])
            pihalf = wpool.tile([P, 1], f32, tag="pihalf", name="pihalf")
            nc.vector.memset(pihalf[:], PI / 2.0)
            # touch Sin once so the ACT table loads during the input DMAs
            warm = wpool.tile([P, 1], bf16, tag="warm", name="warm")
            nc.scalar.activation(warm[:], pihalf[:], AF.Sin)

            ktall = wpool.tile([P, NCH * NCH * P], bf16, tag="ktall",
                               name="ktall")
            nc.scalar.dma_start(ktall[:], ktT[:])
            kt_sb = {}
            for (jc, ic) in nz_pairs:
                o = (jc * NCH + ic) * P
                kt_sb[(jc, ic)] = ktall[:, o:o + P]

            # ---- big input loads (phase path first: it gates the recurrence)
            xall = wpool.tile([P, KD * BL], bf16, tag="xall", name="xall")
            wall = wpool.tile([P, KD * NCH * P], bf16, tag="wall", name="wall")
            for q in range(4):
                xs = q * 2 * BL
                ws = q * 2 * NCH * P
                nc.sync.dma_start(xall[:, xs:xs + 2 * BL],
                                  xbT[:, xs:xs + 2 * BL])
                nc.scalar.dma_start(wall[:, ws:ws + 2 * NCH * P],
                                    wpT[:, ws:ws + 2 * NCH * P])
            xk = [xall[:, k * BL:(k + 1) * BL] for k in range(KD)]
            wk = [wall[:, k * NCH * P:(k + 1) * NCH * P] for k in range(KD)]
            waall = wpool.tile([P, KD * NCH * P], bf16, tag="waall",
                               name="waall")
            nc.gpsimd.dma_start(waall[:], waT[:])
            xfk = xk
            wak = [waall[:, k * NCH * P:(k + 1) * NCH * P] for k in range(KD)]

            boff = [NDUMP * sum(BHS[:h]) for h in range(NH)]
            # ---- per-stream state ----
            phi, cs, mmt, dts, vu = [], [], [], [], []
            ucop = []
            for h in range(NH):
                bh = BHS[h]
                wh = NCH * bh
                phi.append(state_pool.tile([P, wh], bf16, tag=f"phi{h}",
                                           name=f"phi{h}"))
                ucop.append(state_pool.tile([P, wh], bf16, tag=f"ucop{h}",
                                            name=f"ucop{h}"))
                cs.append(state_pool.tile([P, 2 * wh], bf16, tag=f"cs{h}",
                                          name=f"cs{h}"))
                mmt.append(state_pool.tile([P, 2 * wh], bf16, tag=f"mm{h}",
                                           name=f"mm{h}"))
                dts.append([state_pool.tile([P, wh], bf16, tag=f"d{h}_{pb}",
                                            name=f"d{h}_{pb}")
                            for pb in range(2)])
                vu.append(psum.tile([P, 2 * wh], f32, tag=f"vu{h}",
                                    name=f"vu{h}"))
            amp_acc = psum.tile([P, NCH * max(BHS)], f32, tag="ampacc",
                                name="amp_acc")
            pabs = [work.tile([P, NCH * BHS[h]], bf16, tag=f"pabs{h}",
                              name=f"pabs{h}") for h in range(NH)]

            # ---- phase projections -> phi (per stream) ----
            for h in range(NH):
                bh = BHS[h]
                wh = NCH * bh
                for c in range(NCH):
                    acc = vu[h][:, c * bh:(c + 1) * bh]
                    for k in range(KD):
                        nc.tensor.matmul(
                            acc, wk[k][:, c * P:(c + 1) * P],
                            xk[k][:, OFFS[h]:OFFS[h] + bh],
                            start=(k == 0), stop=(k == KD - 1),
                        )
                nc.vector.add_range_wrap(phi[h][:], vu[h][:, 0:wh],
                                         0.0, PI, TWO_PI)
                # pad partitions of chunk 0 carry band-sum taps:
                # 96,97 keep phi=0 (cos=1,sin=0); 98,99 get pi/2 (cos=0,sin=1)
                nc.scalar.dma_start(phi[h][96:100, 0:bh], padphi[:, 0:bh])

            # ---- the recurrence ----
            def emit_amp_path():
                for h in range(NH):
                    bh = BHS[h]
                    wh = NCH * bh
                    for c in range(NCH):
                        acc = amp_acc[:, c * bh:(c + 1) * bh]
                        for k in range(KD):
                            nc.tensor.matmul(
                                acc, wak[k][:, c * P:(c + 1) * P],
                                xfk[k][:, OFFS[h]:OFFS[h] + bh],
                                start=(k == 0), stop=(k == KD - 1),
                            )
                    ab = work.tile([P, wh], f32, tag=f"abs0_{h}",
                                   name=f"abs0_{h}")
                    nc.scalar.activation(ab[:], amp_acc[:, 0:wh], AF.Abs)
                    nc.sync.dma_start(
                        amp0_out[:, aoff[h]:aoff[h] + wh], ab[:])

            amp_at = min(2, MS - 1)
            for it in range(MS):
                if it == amp_at:
                    emit_amp_path()
                for h in range(NH):
                    bh = BHS[h]
                    wh = NCH * bh
                    ph = phi[h]
                    sin = cs[h][:, wh:2 * wh]
                    cos = cs[h][:, 0:wh]
                    # last iteration only feeds the chunk-0 band sums
                    last = (it == MS - 1)
                    cw = bh if last else wh
                    nc.scalar.activation(sin[:, 0:cw], ph[:, 0:cw], AF.Sin)
                    u16 = mybir.dt.uint16
                    nc.vector.tensor_scalar(
                        pabs[h][:, 0:cw].bitcast(u16),
                        ph[:, 0:cw].bitcast(u16),
                        0x7FFF, None, ALU.bitwise_and)
                    nc.scalar.activation(cos[:, 0:cw], pabs[h][:, 0:cw], AF.Sin,
                                         bias=pihalf[:], scale=-1.0)

                    # coupling: [v | u] = (dt*K) [sin | cos]; chunk-0 block
                    # also emits band sums on partitions 96:98
                    for ic in range(NCH):
                        if last and ic > 0:
                            continue
                        jcs = [jc for (jc, i2) in nz_pairs if i2 == ic]
                        for half, srcoff in ((0, wh), (1, 0)):
                            dst = vu[h][:, half * wh + ic * bh:
                                        half * wh + (ic + 1) * bh]
                            for n, jc in enumerate(jcs):
                                src = cs[h][:, srcoff + jc * bh:
                                            srcoff + (jc + 1) * bh]
                                nc.tensor.matmul(
                                    dst, kt_sb[(jc, ic)], src,
                                    start=(n == 0), stop=(n == len(jcs) - 1),
                                )

                    # stash band sums of post-update phase (it >= 1):
                    # vu partitions 96:98, chunk0 of each half -> stash cols
                    if last:
                        # band sums only: mm and d on tap partitions, chunk 0
                        for half in (0, 1):
                            nc.vector.tensor_tensor(
                                mmt[h][96:128, half * wh:half * wh + bh],
                                cs[h][96:128, half * wh:half * wh + bh],
                                vu[h][96:128, half * wh:half * wh + bh],
                                ALU.mult)
                        a, b = (0, wh) if fast_rot else (wh, 0)
                        dtile = dts[h][it % 2]
                        nc.vector.tensor_tensor(
                            dtile[96:100, 0:bh],
                            mmt[h][96:100, a:a + bh],
                            mmt[h][96:100, b:b + bh], ALU.subtract)
                        slot = it if fast_rot else it - 1
                        so = boff[h] + slot * bh
                        nc.sync.dma_start(bs_out[:, so:so + bh],
                                          dtile[96:100, 0:bh])
                        continue

                    # mm = [cos|sin] * [v|u]
                    nc.vector.tensor_tensor(mmt[h][:], cs[h][:], vu[h][:],
                                            ALU.mult)
                    # fast path: d = c*v - s*u (= coup); fallback: d = -coup
                    # since WRAP_SUB computes wrap((phi - d) + s0).
                    # tap partitions 96:100 of chunk 0 hold (Sd, St, -Cd, -Ct)
                    # (negated in fallback mode).
                    dtile = dts[h][it % 2]
                    a, b = (0, wh) if fast_rot else (wh, 0)
                    nc.vector.tensor_tensor(
                        dtile[:], mmt[h][:, a:a + wh],
                        mmt[h][:, b:b + wh], ALU.subtract)
                    if fast_rot or it > 0:
                        slot = it if fast_rot else it - 1
                        so = boff[h] + slot * bh
                        nc.sync.dma_start(bs_out[:, so:so + bh],
                                          dtile[96:100, 0:bh])
                    if fast_rot:
                        if has_res:
                            for c in range(NCH):
                                pe = 96 if c == 0 else P
                                nc.vector.scalar_tensor_tensor(
                                    ph[0:pe, c * bh:(c + 1) * bh],
                                    dtile[0:pe, c * bh:(c + 1) * bh],
                                    dtw_sb[0:pe, c:c + 1],
                                    ph[0:pe, c * bh:(c + 1) * bh],
                                    ALU.add, ALU.add)
                        else:
                            nc.vector.tensor_tensor(
                                ph[0:96, 0:bh], ph[0:96, 0:bh],
                                dtile[0:96, 0:bh], ALU.add)
                            nc.vector.tensor_tensor(
                                ph[:, bh:wh], ph[:, bh:wh],
                                dtile[:, bh:wh], ALU.add)
                    else:
                        for c in range(NCH):
                            pe = 96 if c == 0 else P
                            nc.vector._custom_dve(
                                wrap_sub,
                                out=ph[0:pe, c * bh:(c + 1) * bh],
                                in0=ph[0:pe, c * bh:(c + 1) * bh],
                                in1=dtile[0:pe, c * bh:(c + 1) * bh],
                                s0=dtw_sb[0:pe, c:c + 1],
                                s1=PI,
                                imm2=TWO_PI,
                            )



    nc.compile()
    return nc


def kernel(x, W_phase, W_amp, omega, K):
    import ml_dtypes
    from concourse.bass_utils import run_bass_kernel_spmd

    x = np.asarray(x, dtype=np.float32)
    W_phase = np.asarray(W_phase, dtype=np.float32)
    W_amp = np.asarray(W_amp, dtype=np.float32)
    omega = np.asarray(omega, dtype=np.float32)
    K = np.asarray(K, dtype=np.float32)

    perm = _osc_perm()
    band_of = np.zeros(N_TOTAL, dtype=np.int64)
    band_of[N_DELTA:N_DELTA + N_THETA] = 1
    band_of[N_DELTA + N_THETA:] = 2

    # ---- rotating-frame feasibility ----
    dtww = DT * omega.astype(np.float64)
    A_band = np.array([dtww[band_of == b].mean() for b in range(3)])
    res = dtww - A_band[band_of]                      # per-osc residual
    # coupling drift bound
    row_l1 = DT * np.abs(K.astype(np.float64)).sum(axis=1)
    drift = N_STEPS * (np.abs(res) + row_l1).max()
    # coupled pairs must share a frame rate
    ii, jj = np.nonzero(K)
    frames_ok = np.allclose(A_band[band_of[ii]], A_band[band_of[jj]],
                            rtol=0, atol=1e-12) if len(ii) else True
    fast_rot = bool(frames_ok and drift <= DRIFT_MAX)
    has_res = bool(fast_rot and np.abs(res).max() > 1e-12)

    # ---- host-side packing ----
    wpT = np.zeros((N_DIMS, NCH * P), dtype=ml_dtypes.bfloat16)
    waT = np.zeros((N_DIMS, NCH * P), dtype=np.float32)

    def chunk_pack(a):
        # [N_DIMS, C] -> [128, KD*C] with k-chunks along free dim
        C = a.shape[1]
        return np.ascontiguousarray(
            a.reshape(KD, P, C).transpose(1, 0, 2).reshape(P, KD * C))
    dtw = np.zeros((P, NCH), dtype=np.float32)
    for c in range(NCH):
        n = CHUNK_REAL[c]
        idx = perm[c, :n]
        wpT[:, c * P:c * P + n] = W_phase[idx].T.astype(ml_dtypes.bfloat16)
        waT[:, c * P:c * P + n] = W_amp[idx].T
        if fast_rot:
            dtw[:n, c] = float(STRIDE) * res[idx].astype(np.float32)
        else:
            w = dtww[idx]
            dtw[:n, c] = (np.mod(w + PI, TWO_PI) - PI).astype(np.float32)

    kT = np.zeros((NCH * P, NCH * P), dtype=np.float32)
    for jc in range(NCH):
        nj = CHUNK_REAL[jc]
        jdx = perm[jc, :nj]
        for ic in range(NCH):
            ni = CHUNK_REAL[ic]
            idx = perm[ic, :ni]
            kT[jc * P:jc * P + nj, ic * P:ic * P + ni] = \
                (float(STRIDE) if fast_rot else 1.0) * DT * K[np.ix_(idx, jdx)].T

    nz = [
        (jc, ic)
        for jc in range(NCH)
        for ic in range(NCH)
        if np.any(kT[jc * P:(jc + 1) * P, ic * P:(ic + 1) * P] != 0.0)
    ]
    if (0, 0) not in nz:
        nz.append((0, 0))     # carries the band-sum indicator columns
    for ic in range(1, NCH):
        if not any(i2 == ic for (_, i2) in nz):
            nz.append((ic, ic))
    nz_pairs = tuple(sorted(nz))

    # fuse delta/theta indicator columns into the (0,0) block pads:
    # cols 96,97 tap the sin half (phi_pad=0), cols 98,99 the cos half
    # (phi_pad=pi/2)
    for cc in (96, 98):
        kT[0:N_DELTA, cc] = 1.0
        kT[N_DELTA:96, cc + 1] = 1.0
    ktT = kT.astype(ml_dtypes.bfloat16)

    key = (nz_pairs, fast_rot, has_res)
    if key not in _COMPILED:
        _COMPILED[key] = _build_program(nz_pairs, fast_rot, has_res)
    nc = _COMPILED[key]

    # kt blocks packed [128, (jc*NCH+ic)*128 .. +128]
    ktp = np.zeros((P, NCH * NCH * P), dtype=ml_dtypes.bfloat16)
    for jc in range(NCH):
        for ic in range(NCH):
            ktp[:, (jc * NCH + ic) * P:(jc * NCH + ic + 1) * P] =                 ktT[jc * P:(jc + 1) * P, ic * P:(ic + 1) * P]
    wpp = chunk_pack(wpT.astype(np.float32)).astype(ml_dtypes.bfloat16)
    wap = chunk_pack(waT).astype(ml_dtypes.bfloat16)
    padphi = np.zeros((4, max(BHS)), dtype=ml_dtypes.bfloat16)
    padphi[2:4, :] = np.float32(PI / 2.0)
    in_maps = []
    for i in range(N_CORES):
        xs = x[i * BL:(i + 1) * BL]
        xst = np.ascontiguousarray(xs.T)
        xsp = chunk_pack(xst)
        in_maps.append({
            "xbT": xsp.astype(ml_dtypes.bfloat16),
            "wpT": wpp, "waT": wap, "ktT": ktp, "dtw": dtw,
            "padphi": padphi,
        })

    res_run = run_bass_kernel_spmd(nc, in_maps, core_ids=list(range(N_CORES)))

    # ---- host-side unshard + exact amp reconstruction (f64) ----
    out = np.empty((BATCH, N_TOTAL), dtype=np.float32)
    ks = np.arange(1, N_STEPS + 1, dtype=np.float64)   # stash it index
    # de-rotation phases per band (delta for theta-mod, theta for gamma-mod)
    if fast_rot:
        rotd = ks * A_band[0]
        rott = ks * A_band[1]
    else:
        rotd = np.zeros(N_STEPS)
        rott = np.zeros(N_STEPS)

    NDUMP = (N_STEPS // STRIDE + 1) if fast_rot else N_STEPS
    # map true step k=1..32 to dump index (fast: dump m = sums of phi_{S*m})
    if fast_rot:
        kk = np.arange(1, N_STEPS + 1)
        dmap = kk // STRIDE
    else:
        dmap = np.arange(N_STEPS)
    for i in range(N_CORES):
        r = res_run.results[i]
        amp0v = np.maximum(np.abs(r["amp0"].astype(np.float64)), EPS)
        bsv = r["bsums"].astype(np.float64)      # [4, NDUMP*BL]
        if not fast_rot:
            bsv = -bsv                           # fallback d = -coup sign
        # per-stream decode -> f-factors [BL, N_STEPS, {theta, gamma}]
        f = np.empty((BL, N_STEPS, 2))
        off = 0
        for h in range(NH):
            bh = BHS[h]
            blk = bsv[:, off:off + NDUMP * bh].reshape(4, NDUMP, bh)
            blk = blk[:, dmap]                    # expand to N_STEPS
            S = blk[0:2]                          # [2(d,t), k, j] sin sums
            C = -blk[2:4]
            R = np.sqrt(S * S + C * C)
            R = np.maximum(R, 1e-30)
            # true cos(mean phase) = (C cos(kA) - S sin(kA)) / R
            cd = (C[0] * np.cos(rotd)[:, None]
                  - S[0] * np.sin(rotd)[:, None]) / R[0]
            ct = (C[1] * np.cos(rott)[:, None]
                  - S[1] * np.sin(rott)[:, None]) / R[1]
            sl = slice(OFFS[h], OFFS[h] + bh)
            f[sl, :, 0] = 1.0 + DT * PAC * cd.T   # theta-band factor
            f[sl, :, 1] = 1.0 + DT * PAC * ct.T   # gamma-band factor
            off += NDUMP * bh
        Pk = np.cumprod(f, axis=1)                # [BL, k, 2]
        m = np.minimum.accumulate(Pk, axis=1)
        Pn = Pk[:, -1]                            # [BL, 2]
        mn = m[:, -1]
        Pfac = np.ones((BL, 3))
        Efac = np.ones((BL, 3))
        Pfac[:, 1:] = Pn
        Efac[:, 1:] = Pn / mn
        a0 = np.empty((BL, N_TOTAL))
        ao = 0
        for h in range(NH):
            bh = BHS[h]
            for c in range(NCH):
                n = CHUNK_REAL[c]
                idx = perm[c, :n]
                a0[OFFS[h]:OFFS[h] + bh, idx] =                     amp0v[:n, ao + c * bh:ao + (c + 1) * bh].T
            ao += NCH * bh
        amp = np.maximum(a0 * Pfac[:, band_of], EPS * Efac[:, band_of])
        out[i * BL:(i + 1) * BL] = amp.astype(np.float32)
    return out


# revision 24
# speedup vs baseline: 1.6825x; 1.2245x over previous
"""Trainium2 Bass kernel for DiscreteDeltaThetaGammaLayer.

Coupled Kuramoto-oscillator recurrence:
  phase0 = (x @ W_phase.T) mod 2pi ; amp0 = max(|x @ W_amp.T|, eps)
  32 steps of: intra-band Kuramoto coupling (phase), PAC amplitude modulation
  output: final amp  (4096, 352) f32

Key structural facts exploited (checked on the host, with a full-width
fallback if they don't hold):
  - The output uses only amp0 and the delta/theta band MEAN phases (PAC);
    gamma phases never feed them when K[delta+theta, gamma] == 0 (block-diag
    K), so the phase recurrence runs on the 96 delta+theta oscillators only.
  - Rotating frame per band (phi~ = phi - k*dt*omega_band) removes the
    per-step omega add and the wrap; the Sin LUT is accurate to |x|<~pi+0.65
    and coupling drift is bounded by 32*dt*max|K|_row <= 0.64. The host
    de-rotates the stashed band sums exactly in f64.
  - The band means drift only O(1e-4) under the weak coupling, so the
    coupling is integrated with STRIDE reference-steps per device iteration
    (frozen coupling field), with band sums stashed each iteration and
    host-side nearest-dump expansion. Measured output error ~2e-3 incl. the
    bf16 amp path (tolerance 2e-2).
  - Band sums ride free in the coupling matmul: the K chunk-0 block's pad
    lhsT columns 96:99 carry delta/theta indicators against pinned pad
    phases (0, 0, pi/2, pi/2), so d = mm1-mm2 holds (Sd, St, -Cd, -Ct) on
    partitions 96:100; the stash is a tiny SBUF->DRAM DMA.
  - amp path reuses the bf16 x and bf16 W_amp (error ~2e-3 on output);
    host reconstructs the clamped amp recurrence in closed form (exact).
"""

import math
import sys

sys.path.insert(0, "/opt/trn_rl_repo")

import numpy as np

# ---- problem constants (module hyperparameters) ----
N_DELTA, N_THETA, N_GAMMA = 32, 64, 256
N_TOTAL = 352
N_DIMS = 1024
BATCH = 4096
N_STEPS = 32
DT = 0.01
PAC = 0.3
EPS = 1e-6
TWO_PI = 2.0 * math.pi
PI = math.pi

N_CORES = 8
BL = BATCH // N_CORES          # 512 batch rows per core
BHS = [256, 256]               # independent streams (latency hiding)
OFFS = [0, 256]
NH = len(BHS)
P = 128
NCH = 3                        # oscillator chunks for the amp path
CHUNK_REAL = [96, 128, 128]
KD = N_DIMS // P               # 8 contraction chunks for the projections

LAST_EXEC_NS = None
_COMPILED = {}
_WRAP_SUB = None

# drift budget: |phi~| may reach pi + DRIFT_MAX with Sin LUT err ~1.2e-3
DRIFT_MAX = 0.66
STRIDE = 32                    # reference steps per device iteration


def _osc_perm():
    """orig oscillator index for each (chunk, partition); -1 for pads."""
    perm = -np.ones((NCH, P), dtype=np.int64)
    perm[0, :96] = np.arange(96)           # delta + theta
    perm[1, :] = 96 + np.arange(128)       # gamma 0:128
    perm[2, :] = 224 + np.arange(128)      # gamma 128:256
    return perm


def _get_wrap_sub():
    """Custom DVE op: out = wrap((in0 - in1) + s0) into [-s1, s1], period imm2."""
    global _WRAP_SUB
    if _WRAP_SUB is not None:
        return _WRAP_SUB
    from concourse.dve_spec import C0, C1, C2, Spec, Src0, Src1, lower
    from concourse.dve_uop import DveOpSpec
    import concourse.dve_ops as dvo

    def _ref(in0, in1, s0, s1, imm2):
        y = (in0 - in1) + s0
        return (y + imm2 * ((y < -s1).astype(np.float32)
                            - (y > s1).astype(np.float32))).astype(np.float32)

    _y = (Src0 - Src1) + C0
    spec = Spec(body=_y + C2 * ((_y < -C1) - (_y > C1)), reference=_ref)
    shas = {}
    for ver in ("v3", "v4"):
        tmp = DveOpSpec(name="WRAP_SUB_KERNEL", opcode=31,
                        uops=lower(spec, ver=ver), rd1_en=True)
        shas[ver] = tmp.sha(ver)
    op = dvo.DveOp("WRAP_SUB_KERNEL", spec, subdim=False, uops_sha=shas)
    dvo.OPS.append(op)
    dvo.CUSTOM_DVE_SPECS[op.name] = op.spec
    dvo._SUB_OPCODE_FOR_NAME[op.name] = dvo._CUSTOM_DVE_ROW_BASE + len(dvo.OPS) - 1
    _WRAP_SUB = op
    return op


def _build_program(nz_pairs, fast_rot, has_res, ncp):
    """ncp: number of phase chunks (1 when gamma is output-irrelevant, else 3).
    fast_rot: rotating frame + stride-STRIDE coupling, no wrap.
    Fallback: per-step wrap with dt*omega in s0, stride 1."""
    import concourse.bass as bass
    import concourse.tile as tile
    from concourse import bacc, mybir

    f32 = mybir.dt.float32
    bf16 = mybir.dt.bfloat16
    u16 = mybir.dt.uint16
    AF = mybir.ActivationFunctionType
    ALU = mybir.AluOpType

    wrap_sub = _get_wrap_sub() if not fast_rot else None

    nc = bacc.Bacc("TRN2", target_bir_lowering=False, debug=False)

    # ---- DRAM I/O (host pre-packs k-chunks along the free dim) ----
    xbT = nc.dram_tensor("xbT", [P, KD * BL], bf16, kind="ExternalInput").ap()
    wpT = nc.dram_tensor("wpT", [P, KD * ncp * P], bf16,
                         kind="ExternalInput").ap()
    waT = nc.dram_tensor("waT", [P, KD * NCH * P], bf16,
                         kind="ExternalInput").ap()
    ktT = nc.dram_tensor("ktT", [P, ncp * ncp * P], bf16,
                         kind="ExternalInput").ap()
    # per-(partition,chunk) scalars: residual r (fast path) or dt*omega
    dtw = nc.dram_tensor("dtw", [P, ncp], f32, kind="ExternalInput").ap()
    # tap-partition phase init rows: [0, 0, pi/2, pi/2] x bh
    padphi = nc.dram_tensor("padphi", [4, max(BHS)], bf16,
                            kind="ExternalInput").ap()

    amp0_out = nc.dram_tensor("amp0", [P, NCH * BL], f32,
                              kind="ExternalOutput").ap()
    aoff = [NCH * sum(BHS[:h]) for h in range(NH)]
    MS = (N_STEPS // STRIDE + 1) if fast_rot else (N_STEPS + 1)
    NDUMP = MS if fast_rot else N_STEPS
    # stash: rows (Sd, St, -Cd, -Ct); per stream block of NDUMP*bh cols
    bs_out = nc.dram_tensor("bsums", [4, NDUMP * BL], bf16,
                            kind="ExternalOutput").ap()

    with tile.TileContext(nc) as tc:
        with (
            tc.tile_pool(name="state", bufs=1) as state_pool,
            tc.tile_pool(name="weights", bufs=1) as wpool,
            tc.tile_pool(name="work", bufs=2) as work,
            tc.tile_pool(name="psum", bufs=1, space="PSUM") as psum,
        ):
            # ---- constants; warm the Sin table during the loads ----
            pihalf = wpool.tile([P, 1], f32, tag="pihalf", name="pihalf")
            nc.vector.memset(pihalf[:], PI / 2.0)
            warm = wpool.tile([P, 1], bf16, tag="warm", name="warm")
            nc.scalar.activation(warm[:], pihalf[:], AF.Sin)

            # ---- phase-path loads first: they gate the recurrence ----
            xall = wpool.tile([P, KD * BL], bf16, tag="xall", name="xall")
            wall = wpool.tile([P, KD * ncp * P], bf16, tag="wall", name="wall")
            wq = KD * ncp * P // 4
            for q in range(4):
                nc.sync.dma_start(xall[:, q * 2 * BL:(q + 1) * 2 * BL],
                                  xbT[:, q * 2 * BL:(q + 1) * 2 * BL])
                nc.scalar.dma_start(wall[:, q * wq:(q + 1) * wq],
                                    wpT[:, q * wq:(q + 1) * wq])
            xk = [xall[:, k * BL:(k + 1) * BL] for k in range(KD)]
            wk = [wall[:, k * ncp * P:(k + 1) * ncp * P] for k in range(KD)]

            ktall = wpool.tile([P, ncp * ncp * P], bf16, tag="ktall",
                               name="ktall")
            nc.scalar.dma_start(ktall[:], ktT[:])
            kt_sb = {}
            for (jc, ic) in nz_pairs:
                o = (jc * ncp + ic) * P
                kt_sb[(jc, ic)] = ktall[:, o:o + P]
            dtw_sb = None
            if (not fast_rot) or has_res:
                dtw_sb = wpool.tile([P, ncp], f32, tag="dtw", name="dtw_sb")
                nc.scalar.dma_start(dtw_sb[:], dtw[:])

            # amp weights are lazy (gpsimd SWDGE, off the critical path)
            waall = wpool.tile([P, KD * NCH * P], bf16, tag="waall",
                               name="waall")
            nc.gpsimd.dma_start(waall[:], waT[:])
            wak = [waall[:, k * NCH * P:(k + 1) * NCH * P] for k in range(KD)]

            # ---- per-stream state (phase width = ncp*bh) ----
            boff = [NDUMP * sum(BHS[:h]) for h in range(NH)]
            phi, cs, mmt, dts, pabs, vu = [], [], [], [], [], []
            for h in range(NH):
                bh = BHS[h]
                wh = ncp * bh
                phi.append(state_pool.tile([P, wh], bf16, tag=f"phi{h}",
                                           name=f"phi{h}"))
                cs.append(state_pool.tile([P, 2 * wh], bf16, tag=f"cs{h}",
                                          name=f"cs{h}"))
                mmt.append(state_pool.tile([P, 2 * wh], bf16, tag=f"mm{h}",
                                           name=f"mm{h}"))
                dts.append([state_pool.tile([P, wh], bf16, tag=f"d{h}_{pb}",
                                            name=f"d{h}_{pb}")
                            for pb in range(2)])
                pabs.append(work.tile([P, wh], bf16, tag=f"pabs{h}",
                                      name=f"pabs{h}"))
                vu.append(psum.tile([P, 2 * wh], f32, tag=f"vu{h}",
                                    name=f"vu{h}"))
            amp_acc = psum.tile([P, NCH * max(BHS)], f32, tag="ampacc",
                                name="amp_acc")

            # ---- phase projections -> phi (per stream) ----
            for h in range(NH):
                bh = BHS[h]
                wh = ncp * bh
                for c in range(ncp):
                    acc = vu[h][:, c * bh:(c + 1) * bh]
                    for k in range(KD):
                        nc.tensor.matmul(
                            acc, wk[k][:, c * P:(c + 1) * P],
                            xk[k][:, OFFS[h]:OFFS[h] + bh],
                            start=(k == 0), stop=(k == KD - 1),
                        )
                nc.vector.add_range_wrap(phi[h][:], vu[h][:, 0:wh],
                                         0.0, PI, TWO_PI)
                # pad partitions of chunk 0 carry band-sum taps:
                # 96,97 keep phi=0 (cos=1,sin=0); 98,99 pi/2 (cos=0,sin=1)
                nc.scalar.dma_start(phi[h][96:100, 0:bh], padphi[:, 0:bh])

            # ---- amp path (emitted mid-loop so PE fills its gaps) ----
            def emit_amp_path():
                for h in range(NH):
                    bh = BHS[h]
                    awh = NCH * bh
                    for c in range(NCH):
                        acc = amp_acc[:, c * bh:(c + 1) * bh]
                        for k in range(KD):
                            nc.tensor.matmul(
                                acc, wak[k][:, c * P:(c + 1) * P],
                                xk[k][:, OFFS[h]:OFFS[h] + bh],
                                start=(k == 0), stop=(k == KD - 1),
                            )
                    ab = work.tile([P, awh], f32, tag=f"abs0_{h}",
                                   name=f"abs0_{h}")
                    nc.scalar.activation(ab[:], amp_acc[:, 0:awh], AF.Abs)
                    nc.sync.dma_start(
                        amp0_out[:, aoff[h]:aoff[h] + awh], ab[:])

            # ---- the recurrence ----
            amp_at = min(1, MS - 1)
            for it in range(MS):
                if it == amp_at:
                    emit_amp_path()
                for h in range(NH):
                    bh = BHS[h]
                    wh = ncp * bh
                    ph = phi[h]
                    sin = cs[h][:, wh:2 * wh]
                    cos = cs[h][:, 0:wh]
                    last = (it == MS - 1)
                    nc.scalar.activation(sin[:], ph[:], AF.Sin)
                    nc.vector.tensor_scalar(
                        pabs[h][:].bitcast(u16), ph[:].bitcast(u16),
                        0x7FFF, None, ALU.bitwise_and)
                    nc.scalar.activation(cos[:], pabs[h][:], AF.Sin,
                                         bias=pihalf[:], scale=-1.0)

                    # coupling: [v | u] = (S*dt*K) [sin | cos]; chunk-0 block
                    # also emits band sums on partitions 96:100
                    for ic in range(ncp):
                        jcs = [jc for (jc, i2) in nz_pairs if i2 == ic]
                        for half, srcoff in ((0, wh), (1, 0)):
                            dst = vu[h][:, half * wh + ic * bh:
                                        half * wh + (ic + 1) * bh]
                            for n, jc in enumerate(jcs):
                                src = cs[h][:, srcoff + jc * bh:
                                            srcoff + (jc + 1) * bh]
                                nc.tensor.matmul(
                                    dst, kt_sb[(jc, ic)], src,
                                    start=(n == 0), stop=(n == len(jcs) - 1),
                                )

                    # mm = [cos|sin] * [v|u]; d = c*v - s*u (fast) or -coup
                    # (fallback, for WRAP_SUB's wrap((phi - d) + s0)).
                    # d partitions 96:100 hold (Sd, St, -Cd, -Ct).
                    dtile = dts[h][it % 2]
                    nc.vector.tensor_tensor(mmt[h][:], cs[h][:], vu[h][:],
                                            ALU.mult)
                    a, b = (0, wh) if fast_rot else (wh, 0)
                    nc.vector.tensor_tensor(
                        dtile[:], mmt[h][:, a:a + wh],
                        mmt[h][:, b:b + wh], ALU.subtract)
                    if fast_rot or it > 0:
                        slot = it if fast_rot else it - 1
                        so = boff[h] + slot * bh
                        nc.sync.dma_start(bs_out[:, so:so + bh],
                                          dtile[96:100, 0:bh])
                    if last:
                        continue

                    # phi update (tap partitions 96:100 excluded on chunk 0)
                    if fast_rot:
                        if has_res:
                            for c in range(ncp):
                                pe = 96 if c == 0 else P
                                nc.vector.scalar_tensor_tensor(
                                    ph[0:pe, c * bh:(c + 1) * bh],
                                    dtile[0:pe, c * bh:(c + 1) * bh],
                                    dtw_sb[0:pe, c:c + 1],
                                    ph[0:pe, c * bh:(c + 1) * bh],
                                    ALU.add, ALU.add)
                        else:
                            nc.vector.tensor_tensor(
                                ph[0:96, 0:bh], ph[0:96, 0:bh],
                                dtile[0:96, 0:bh], ALU.add)
                            if ncp > 1:
                                nc.vector.tensor_tensor(
                                    ph[:, bh:wh], ph[:, bh:wh],
                                    dtile[:, bh:wh], ALU.add)
                    else:
                        for c in range(ncp):
                            pe = 96 if c == 0 else P
                            nc.vector._custom_dve(
                                wrap_sub,
                                out=ph[0:pe, c * bh:(c + 1) * bh],
                                in0=ph[0:pe, c * bh:(c + 1) * bh],
                                in1=dtile[0:pe, c * bh:(c + 1) * bh],
                                s0=dtw_sb[0:pe, c:c + 1],
                                s1=PI,
                                imm2=TWO_PI,
                            )

    nc.compile()
    return nc


def kernel(x, W_phase, W_amp, omega, K):
    import ml_dtypes
    from concourse.bass_utils import run_bass_kernel_spmd

    x = np.asarray(x, dtype=np.float32)
    W_phase = np.asarray(W_phase, dtype=np.float32)
    W_amp = np.asarray(W_amp, dtype=np.float32)
    omega = np.asarray(omega, dtype=np.float32)
    K = np.asarray(K, dtype=np.float32)

    perm = _osc_perm()
    band_of = np.zeros(N_TOTAL, dtype=np.int64)
    band_of[N_DELTA:N_DELTA + N_THETA] = 1
    band_of[N_DELTA + N_THETA:] = 2

    # ---- structural checks ----
    Kf = K.astype(np.float64)
    dtww = DT * omega.astype(np.float64)
    A_band = np.array([dtww[band_of == b].mean() for b in range(3)])
    res = dtww - A_band[band_of]
    row_l1 = DT * np.abs(Kf).sum(axis=1)
    drift = N_STEPS * (np.abs(res) + row_l1).max()
    ii, jj = np.nonzero(K)
    frames_ok = np.allclose(A_band[band_of[ii]], A_band[band_of[jj]],
                            rtol=0, atol=1e-12) if len(ii) else True
    fast_rot = bool(frames_ok and drift <= DRIFT_MAX)
    has_res = bool(fast_rot and np.abs(res).max() > 1e-12)
    # gamma is output-irrelevant iff it never couples into delta/theta
    g_isolated = not np.any(Kf[0:96, 96:] != 0.0)
    ncp = 1 if g_isolated else NCH

    # ---- host-side packing ----
    def chunk_pack(a):
        # [N_DIMS, C] -> [128, KD*C] with k-chunks along free dim
        C = a.shape[1]
        return np.ascontiguousarray(
            a.reshape(KD, P, C).transpose(1, 0, 2).reshape(P, KD * C))

    wpT = np.zeros((N_DIMS, ncp * P), dtype=np.float32)
    waT = np.zeros((N_DIMS, NCH * P), dtype=np.float32)
    dtw = np.zeros((P, ncp), dtype=np.float32)
    for c in range(ncp):
        n = CHUNK_REAL[c]
        idx = perm[c, :n]
        wpT[:, c * P:c * P + n] = W_phase[idx].T
        if fast_rot:
            dtw[:n, c] = float(STRIDE) * res[idx].astype(np.float32)
        else:
            w = dtww[idx]
            dtw[:n, c] = (np.mod(w + PI, TWO_PI) - PI).astype(np.float32)
    for c in range(NCH):
        n = CHUNK_REAL[c]
        idx = perm[c, :n]
        waT[:, c * P:c * P + n] = W_amp[idx].T

    kT = np.zeros((ncp * P, ncp * P), dtype=np.float32)
    for jc in range(ncp):
        nj = CHUNK_REAL[jc]
        jdx = perm[jc, :nj]
        for ic in range(ncp):
            ni = CHUNK_REAL[ic]
            idx = perm[ic, :ni]
            kT[jc * P:jc * P + nj, ic * P:ic * P + ni] = \
                (float(STRIDE) if fast_rot else 1.0) * DT * \
                K[np.ix_(idx, jdx)].T

    nz = [
        (jc, ic)
        for jc in range(ncp)
        for ic in range(ncp)
        if np.any(kT[jc * P:(jc + 1) * P, ic * P:(ic + 1) * P] != 0.0)
    ]
    if (0, 0) not in nz:
        nz.append((0, 0))     # carries the band-sum indicator columns
    for ic in range(1, ncp):
        if not any(i2 == ic for (_, i2) in nz):
            nz.append((ic, ic))
    nz_pairs = tuple(sorted(nz))

    # fuse delta/theta indicator columns into the (0,0) block pads:
    # cols 96,97 tap the sin half (phi_pad=0), 98,99 the cos half (pi/2)
    for cc in (96, 98):
        kT[0:N_DELTA, cc] = 1.0
        kT[N_DELTA:96, cc + 1] = 1.0

    key = (nz_pairs, fast_rot, has_res, ncp)
    if key not in _COMPILED:
        _COMPILED[key] = _build_program(nz_pairs, fast_rot, has_res, ncp)
    nc = _COMPILED[key]

    # kt blocks packed [128, (jc*ncp+ic)*128 .. +128]
    ktp = np.zeros((P, ncp * ncp * P), dtype=ml_dtypes.bfloat16)
    for jc in range(ncp):
        for ic in range(ncp):
            ktp[:, (jc * ncp + ic) * P:(jc * ncp + ic + 1) * P] = \
                kT[jc * P:(jc + 1) * P, ic * P:(ic + 1) * P]
    wpp = chunk_pack(wpT).astype(ml_dtypes.bfloat16)
    wap = chunk_pack(waT).astype(ml_dtypes.bfloat16)
    padphi = np.zeros((4, max(BHS)), dtype=ml_dtypes.bfloat16)
    padphi[2:4, :] = np.float32(PI / 2.0)
    in_maps = []
    for i in range(N_CORES):
        xs = x[i * BL:(i + 1) * BL]
        xsp = chunk_pack(np.ascontiguousarray(xs.T))
        in_maps.append({
            "xbT": xsp.astype(ml_dtypes.bfloat16),
            "wpT": wpp, "waT": wap, "ktT": ktp, "dtw": dtw,
            "padphi": padphi,
        })

    res_run = run_bass_kernel_spmd(nc, in_maps, core_ids=list(range(N_CORES)))

    # ---- host-side unshard + exact amp reconstruction (f64) ----
    out = np.empty((BATCH, N_TOTAL), dtype=np.float32)
    NDUMP = (N_STEPS // STRIDE + 1) if fast_rot else N_STEPS
    kk = np.arange(1, N_STEPS + 1)
    dmap = (kk // STRIDE) if fast_rot else (kk - 1)
    ks = kk.astype(np.float64)
    if fast_rot:
        rotd = ks * A_band[0]
        rott = ks * A_band[1]
    else:
        rotd = np.zeros(N_STEPS)
        rott = np.zeros(N_STEPS)

    for i in range(N_CORES):
        r = res_run.results[i]
        amp0v = np.maximum(np.abs(r["amp0"].astype(np.float64)), EPS)
        bsv = r["bsums"].astype(np.float64)      # [4, NDUMP*BL]
        if not fast_rot:
            bsv = -bsv                           # fallback d = -coup sign
        f = np.empty((BL, N_STEPS, 2))
        off = 0
        for h in range(NH):
            bh = BHS[h]
            blk = bsv[:, off:off + NDUMP * bh].reshape(4, NDUMP, bh)
            blk = blk[:, dmap]                    # expand to N_STEPS
            S = blk[0:2]                          # [2(d,t), k, j] sin sums
            C = -blk[2:4]
            R = np.sqrt(S * S + C * C)
            R = np.maximum(R, 1e-30)
            cd = (C[0] * np.cos(rotd)[:, None]
                  - S[0] * np.sin(rotd)[:, None]) / R[0]
            ct = (C[1] * np.cos(rott)[:, None]
                  - S[1] * np.sin(rott)[:, None]) / R[1]
            sl = slice(OFFS[h], OFFS[h] + bh)
            f[sl, :, 0] = 1.0 + DT * PAC * cd.T   # theta-band factor
            f[sl, :, 1] = 1.0 + DT * PAC * ct.T   # gamma-band factor
            off += NDUMP * bh
        Pk = np.cumprod(f, axis=1)
        m = np.minimum.accumulate(Pk, axis=1)
        Pn = Pk[:, -1]
        mn = m[:, -1]
        Pfac = np.ones((BL, 3))
        Efac = np.ones((BL, 3))
        Pfac[:, 1:] = Pn
        Efac[:, 1:] = Pn / mn
        a0 = np.empty((BL, N_TOTAL))
        ao = 0
        for h in range(NH):
            bh = BHS[h]
            for c in range(NCH):
                n = CHUNK_REAL[c]
                idx = perm[c, :n]
                a0[OFFS[h]:OFFS[h] + bh, idx] = \
                    amp0v[:n, ao + c * bh:ao + (c + 1) * bh].T
            ao += NCH * bh
        amp = np.maximum(a0 * Pfac[:, band_of], EPS * Efac[:, band_of])
        out[i * BL:(i + 1) * BL] = amp.astype(np.float32)
    return out


# revision 25
# speedup vs baseline: 1.7991x; 1.0693x over previous
"""Trainium2 Bass kernel for DiscreteDeltaThetaGammaLayer.

Coupled Kuramoto-oscillator recurrence:
  phase0 = (x @ W_phase.T) mod 2pi ; amp0 = max(|x @ W_amp.T|, eps)
  32 steps of: intra-band Kuramoto coupling (phase), PAC amplitude modulation
  output: final amp  (4096, 352) f32

Key structural facts exploited (checked on the host, with a full-width
fallback if they don't hold):
  - The output uses only amp0 and the delta/theta band MEAN phases (PAC);
    gamma phases never feed them when K[delta+theta, gamma] == 0 (block-diag
    K), so the phase recurrence runs on the 96 delta+theta oscillators only.
  - Rotating frame per band (phi~ = phi - k*dt*omega_band) removes the
    per-step omega add and the wrap; the Sin LUT is accurate to |x|<~pi+0.65
    and coupling drift is bounded by 32*dt*max|K|_row <= 0.64. The host
    de-rotates the stashed band sums exactly in f64.
  - The band means drift only O(1e-4) under the weak coupling, so the
    coupling is integrated with STRIDE reference-steps per device iteration
    (frozen coupling field), with band sums stashed each iteration and
    host-side nearest-dump expansion. Measured output error ~2e-3 incl. the
    bf16 amp path (tolerance 2e-2).
  - Band sums ride free in the coupling matmul: the K chunk-0 block's pad
    lhsT columns 96:99 carry delta/theta indicators against pinned pad
    phases (0, 0, pi/2, pi/2), so d = mm1-mm2 holds (Sd, St, -Cd, -Ct) on
    partitions 96:100; the stash is a tiny SBUF->DRAM DMA.
  - amp path reuses the bf16 x and bf16 W_amp (error ~2e-3 on output);
    host reconstructs the clamped amp recurrence in closed form (exact).
"""

import math
import sys

sys.path.insert(0, "/opt/trn_rl_repo")

import numpy as np

# ---- problem constants (module hyperparameters) ----
N_DELTA, N_THETA, N_GAMMA = 32, 64, 256
N_TOTAL = 352
N_DIMS = 1024
BATCH = 4096
N_STEPS = 32
DT = 0.01
PAC = 0.3
EPS = 1e-6
TWO_PI = 2.0 * math.pi
PI = math.pi

N_CORES = 8
BL = BATCH // N_CORES          # 512 batch rows per core
BHS = [256, 256]               # independent streams (latency hiding)
OFFS = [0, 256]
NH = len(BHS)
P = 128
NCH = 3                        # oscillator chunks for the amp path
CHUNK_REAL = [96, 128, 128]
KD = N_DIMS // P               # 8 contraction chunks for the projections

LAST_EXEC_NS = None
_COMPILED = {}
_WRAP_SUB = None

# drift budget: |phi~| may reach pi + DRIFT_MAX with Sin LUT err ~1.2e-3
DRIFT_MAX = 0.66
STRIDE = 32                    # reference steps per device iteration


def _osc_perm():
    """orig oscillator index for each (chunk, partition); -1 for pads."""
    perm = -np.ones((NCH, P), dtype=np.int64)
    perm[0, :96] = np.arange(96)           # delta + theta
    perm[1, :] = 96 + np.arange(128)       # gamma 0:128
    perm[2, :] = 224 + np.arange(128)      # gamma 128:256
    return perm


def _get_wrap_sub():
    """Custom DVE op: out = wrap((in0 - in1) + s0) into [-s1, s1], period imm2."""
    global _WRAP_SUB
    if _WRAP_SUB is not None:
        return _WRAP_SUB
    from concourse.dve_spec import C0, C1, C2, Spec, Src0, Src1, lower
    from concourse.dve_uop import DveOpSpec
    import concourse.dve_ops as dvo

    def _ref(in0, in1, s0, s1, imm2):
        y = (in0 - in1) + s0
        return (y + imm2 * ((y < -s1).astype(np.float32)
                            - (y > s1).astype(np.float32))).astype(np.float32)

    _y = (Src0 - Src1) + C0
    spec = Spec(body=_y + C2 * ((_y < -C1) - (_y > C1)), reference=_ref)
    shas = {}
    for ver in ("v3", "v4"):
        tmp = DveOpSpec(name="WRAP_SUB_KERNEL", opcode=31,
                        uops=lower(spec, ver=ver), rd1_en=True)
        shas[ver] = tmp.sha(ver)
    op = dvo.DveOp("WRAP_SUB_KERNEL", spec, subdim=False, uops_sha=shas)
    dvo.OPS.append(op)
    dvo.CUSTOM_DVE_SPECS[op.name] = op.spec
    dvo._SUB_OPCODE_FOR_NAME[op.name] = dvo._CUSTOM_DVE_ROW_BASE + len(dvo.OPS) - 1
    _WRAP_SUB = op
    return op


def _build_program(nz_pairs, fast_rot, has_res, ncp):
    """ncp: number of phase chunks (1 when gamma is output-irrelevant, else 3).
    fast_rot: rotating frame + stride-STRIDE coupling, no wrap.
    Fallback: per-step wrap with dt*omega in s0, stride 1."""
    import concourse.bass as bass
    import concourse.tile as tile
    from concourse import bacc, mybir

    f32 = mybir.dt.float32
    bf16 = mybir.dt.bfloat16
    u16 = mybir.dt.uint16
    AF = mybir.ActivationFunctionType
    ALU = mybir.AluOpType

    wrap_sub = _get_wrap_sub() if not fast_rot else None

    nc = bacc.Bacc("TRN2", target_bir_lowering=False, debug=False)

    # ---- DRAM I/O (host pre-packs k-chunks along the free dim) ----
    xbT = nc.dram_tensor("xbT", [P, KD * BL], bf16, kind="ExternalInput").ap()
    wpT = nc.dram_tensor("wpT", [P, KD * ncp * P], bf16,
                         kind="ExternalInput").ap()
    waT = nc.dram_tensor("waT", [P, KD * NCH * P], bf16,
                         kind="ExternalInput").ap()
    ktT = nc.dram_tensor("ktT", [P, ncp * ncp * P], bf16,
                         kind="ExternalInput").ap()
    # per-(partition,chunk) scalars: residual r (fast path) or dt*omega
    dtw = nc.dram_tensor("dtw", [P, ncp], f32, kind="ExternalInput").ap()
    # tap-partition phase init rows: [0, 0, pi/2, pi/2] x bh
    padphi = nc.dram_tensor("padphi", [4, max(BHS)], bf16,
                            kind="ExternalInput").ap()

    amp0_out = nc.dram_tensor("amp0", [P, NCH * BL], f32,
                              kind="ExternalOutput").ap()
    aoff = [NCH * sum(BHS[:h]) for h in range(NH)]
    MS = (N_STEPS // STRIDE + 1) if fast_rot else (N_STEPS + 1)
    NDUMP = MS if fast_rot else N_STEPS
    # stash: rows (Sd, St, -Cd, -Ct); per stream block of NDUMP*bh cols
    bs_out = nc.dram_tensor("bsums", [4, NDUMP * BL], bf16,
                            kind="ExternalOutput").ap()

    with tile.TileContext(nc) as tc:
        with (
            tc.tile_pool(name="state", bufs=1) as state_pool,
            tc.tile_pool(name="weights", bufs=1) as wpool,
            tc.tile_pool(name="work", bufs=2) as work,
            tc.tile_pool(name="psum", bufs=1, space="PSUM") as psum,
        ):
            # ---- constants; warm the Sin table during the loads ----
            pihalf = wpool.tile([P, 1], f32, tag="pihalf", name="pihalf")
            nc.vector.memset(pihalf[:], PI / 2.0)
            warm = wpool.tile([P, 1], bf16, tag="warm", name="warm")
            nc.scalar.activation(warm[:], pihalf[:], AF.Sin)

            # ---- phase-path loads first: they gate the recurrence ----
            xall = wpool.tile([P, KD * BL], bf16, tag="xall", name="xall")
            wall = wpool.tile([P, KD * ncp * P], bf16, tag="wall", name="wall")
            wq = KD * ncp * P // 4
            for q in range(4):
                nc.sync.dma_start(xall[:, q * 2 * BL:(q + 1) * 2 * BL],
                                  xbT[:, q * 2 * BL:(q + 1) * 2 * BL])
                nc.scalar.dma_start(wall[:, q * wq:(q + 1) * wq],
                                    wpT[:, q * wq:(q + 1) * wq])
            xk = [xall[:, k * BL:(k + 1) * BL] for k in range(KD)]
            wk = [wall[:, k * ncp * P:(k + 1) * ncp * P] for k in range(KD)]

            ktall = wpool.tile([P, ncp * ncp * P], bf16, tag="ktall",
                               name="ktall")
            nc.scalar.dma_start(ktall[:], ktT[:])
            kt_sb = {}
            for (jc, ic) in nz_pairs:
                o = (jc * ncp + ic) * P
                kt_sb[(jc, ic)] = ktall[:, o:o + P]
            dtw_sb = None
            if (not fast_rot) or has_res:
                dtw_sb = wpool.tile([P, ncp], f32, tag="dtw", name="dtw_sb")
                nc.scalar.dma_start(dtw_sb[:], dtw[:])

            # amp weights load after the phase inputs on the sync queue
            waall = wpool.tile([P, KD * NCH * P], bf16, tag="waall",
                               name="waall")
            nc.sync.dma_start(waall[:], waT[:])
            wak = [waall[:, k * NCH * P:(k + 1) * NCH * P] for k in range(KD)]

            # ---- per-stream state (phase width = ncp*bh) ----
            boff = [NDUMP * sum(BHS[:h]) for h in range(NH)]
            phi, cs, mmt, dts, pabs, vu = [], [], [], [], [], []
            for h in range(NH):
                bh = BHS[h]
                wh = ncp * bh
                phi.append(state_pool.tile([P, wh], bf16, tag=f"phi{h}",
                                           name=f"phi{h}"))
                cs.append(state_pool.tile([P, 2 * wh], bf16, tag=f"cs{h}",
                                          name=f"cs{h}"))
                mmt.append(state_pool.tile([P, 2 * wh], bf16, tag=f"mm{h}",
                                           name=f"mm{h}"))
                dts.append([state_pool.tile([P, wh], bf16, tag=f"d{h}_{pb}",
                                            name=f"d{h}_{pb}")
                            for pb in range(2)])
                pabs.append(work.tile([P, wh], bf16, tag=f"pabs{h}",
                                      name=f"pabs{h}"))
                vu.append(psum.tile([P, 2 * wh], f32, tag=f"vu{h}",
                                    name=f"vu{h}"))
                # tap partitions: 96,97 phi=0 (cos=1,sin=0); 98,99 pi/2;
                # 100:128 zeroed. wrap later writes partitions 0:96 only.
                nc.vector.memset(phi[h][96:128, :], 0.0)
                nc.scalar.dma_start(phi[h][96:100, 0:bh], padphi[:, 0:bh])
            amp_acc = psum.tile([P, NCH * max(BHS)], f32, tag="ampacc",
                                name="amp_acc")

            # ---- phase projections -> phi (per stream) ----
            for h in range(NH):
                bh = BHS[h]
                wh = ncp * bh
                for c in range(ncp):
                    acc = vu[h][:, c * bh:(c + 1) * bh]
                    for k in range(KD):
                        nc.tensor.matmul(
                            acc, wk[k][:, c * P:(c + 1) * P],
                            xk[k][:, OFFS[h]:OFFS[h] + bh],
                            start=(k == 0), stop=(k == KD - 1),
                        )
                nc.vector.add_range_wrap(phi[h][0:96, 0:bh],
                                         vu[h][0:96, 0:bh], 0.0, PI, TWO_PI)
                if ncp > 1:
                    nc.vector.add_range_wrap(phi[h][:, bh:wh],
                                             vu[h][:, bh:wh], 0.0, PI, TWO_PI)

            # ---- amp path (emitted mid-loop so PE fills its gaps) ----
            def emit_amp_path(h):
                bh = BHS[h]
                awh = NCH * bh
                for c in range(NCH):
                    acc = amp_acc[:, c * bh:(c + 1) * bh]
                    for k in range(KD):
                        nc.tensor.matmul(
                            acc, wak[k][:, c * P:(c + 1) * P],
                            xk[k][:, OFFS[h]:OFFS[h] + bh],
                            start=(k == 0), stop=(k == KD - 1),
                        )
                ab = work.tile([P, awh], f32, tag=f"abs0_{h}",
                               name=f"abs0_{h}")
                nc.scalar.activation(ab[:], amp_acc[:, 0:awh], AF.Abs)
                nc.scalar.dma_start(
                    amp0_out[:, aoff[h]:aoff[h] + awh], ab[:])

            # ---- the recurrence ----
            amp_at = min(1, MS - 1)
            for it in range(MS):
                if it == amp_at:
                    emit_amp_path(0)
                for h in range(NH):
                    bh = BHS[h]
                    wh = ncp * bh
                    ph = phi[h]
                    sin = cs[h][:, wh:2 * wh]
                    cos = cs[h][:, 0:wh]
                    last = (it == MS - 1)
                    nc.scalar.activation(sin[:], ph[:], AF.Sin)
                    nc.vector.tensor_scalar(
                        pabs[h][:].bitcast(u16), ph[:].bitcast(u16),
                        0x7FFF, None, ALU.bitwise_and)
                    nc.scalar.activation(cos[:], pabs[h][:], AF.Sin,
                                         bias=pihalf[:], scale=-1.0)

                    # coupling: [v | u] = (S*dt*K) [sin | cos]; chunk-0 block
                    # also emits band sums on partitions 96:100
                    for ic in range(ncp):
                        jcs = [jc for (jc, i2) in nz_pairs if i2 == ic]
                        for half, srcoff in ((0, wh), (1, 0)):
                            dst = vu[h][:, half * wh + ic * bh:
                                        half * wh + (ic + 1) * bh]
                            for n, jc in enumerate(jcs):
                                src = cs[h][:, srcoff + jc * bh:
                                            srcoff + (jc + 1) * bh]
                                nc.tensor.matmul(
                                    dst, kt_sb[(jc, ic)], src,
                                    start=(n == 0), stop=(n == len(jcs) - 1),
                                )

                    # mm = [cos|sin] * [v|u]; d = c*v - s*u (fast) or -coup
                    # (fallback, for WRAP_SUB's wrap((phi - d) + s0)).
                    # d partitions 96:100 hold (Sd, St, -Cd, -Ct).
                    dtile = dts[h][it % 2]
                    nc.vector.tensor_tensor(mmt[h][:], cs[h][:], vu[h][:],
                                            ALU.mult)
                    a, b = (0, wh) if fast_rot else (wh, 0)
                    nc.vector.tensor_tensor(
                        dtile[:], mmt[h][:, a:a + wh],
                        mmt[h][:, b:b + wh], ALU.subtract)
                    if fast_rot or it > 0:
                        slot = it if fast_rot else it - 1
                        so = boff[h] + slot * bh
                        nc.sync.dma_start(bs_out[:, so:so + bh],
                                          dtile[96:100, 0:bh])
                    if last:
                        continue

                    # phi update (tap partitions 96:100 excluded on chunk 0)
                    if fast_rot:
                        if has_res:
                            for c in range(ncp):
                                pe = 96 if c == 0 else P
                                nc.vector.scalar_tensor_tensor(
                                    ph[0:pe, c * bh:(c + 1) * bh],
                                    dtile[0:pe, c * bh:(c + 1) * bh],
                                    dtw_sb[0:pe, c:c + 1],
                                    ph[0:pe, c * bh:(c + 1) * bh],
                                    ALU.add, ALU.add)
                        else:
                            nc.vector.tensor_tensor(
                                ph[0:96, 0:bh], ph[0:96, 0:bh],
                                dtile[0:96, 0:bh], ALU.add)
                            if ncp > 1:
                                nc.vector.tensor_tensor(
                                    ph[:, bh:wh], ph[:, bh:wh],
                                    dtile[:, bh:wh], ALU.add)
                    else:
                        for c in range(ncp):
                            pe = 96 if c == 0 else P
                            nc.vector._custom_dve(
                                wrap_sub,
                                out=ph[0:pe, c * bh:(c + 1) * bh],
                                in0=ph[0:pe, c * bh:(c + 1) * bh],
                                in1=dtile[0:pe, c * bh:(c + 1) * bh],
                                s0=dtw_sb[0:pe, c:c + 1],
                                s1=PI,
                                imm2=TWO_PI,
                            )

            emit_amp_path(1)

    nc.compile()
    return nc


def kernel(x, W_phase, W_amp, omega, K):
    import ml_dtypes
    from concourse.bass_utils import run_bass_kernel_spmd

    x = np.asarray(x, dtype=np.float32)
    W_phase = np.asarray(W_phase, dtype=np.float32)
    W_amp = np.asarray(W_amp, dtype=np.float32)
    omega = np.asarray(omega, dtype=np.float32)
    K = np.asarray(K, dtype=np.float32)

    perm = _osc_perm()
    band_of = np.zeros(N_TOTAL, dtype=np.int64)
    band_of[N_DELTA:N_DELTA + N_THETA] = 1
    band_of[N_DELTA + N_THETA:] = 2

    # ---- structural checks ----
    Kf = K.astype(np.float64)
    dtww = DT * omega.astype(np.float64)
    A_band = np.array([dtww[band_of == b].mean() for b in range(3)])
    res = dtww - A_band[band_of]
    row_l1 = DT * np.abs(Kf).sum(axis=1)
    drift = N_STEPS * (np.abs(res) + row_l1).max()
    ii, jj = np.nonzero(K)
    frames_ok = np.allclose(A_band[band_of[ii]], A_band[band_of[jj]],
                            rtol=0, atol=1e-12) if len(ii) else True
    fast_rot = bool(frames_ok and drift <= DRIFT_MAX)
    has_res = bool(fast_rot and np.abs(res).max() > 1e-12)
    # gamma is output-irrelevant iff it never couples into delta/theta
    g_isolated = not np.any(Kf[0:96, 96:] != 0.0)
    ncp = 1 if g_isolated else NCH

    # ---- host-side packing ----
    def chunk_pack(a):
        # [N_DIMS, C] -> [128, KD*C] with k-chunks along free dim
        C = a.shape[1]
        return np.ascontiguousarray(
            a.reshape(KD, P, C).transpose(1, 0, 2).reshape(P, KD * C))

    wpT = np.zeros((N_DIMS, ncp * P), dtype=np.float32)
    waT = np.zeros((N_DIMS, NCH * P), dtype=np.float32)
    dtw = np.zeros((P, ncp), dtype=np.float32)
    for c in range(ncp):
        n = CHUNK_REAL[c]
        idx = perm[c, :n]
        wpT[:, c * P:c * P + n] = W_phase[idx].T
        if fast_rot:
            dtw[:n, c] = float(STRIDE) * res[idx].astype(np.float32)
        else:
            w = dtww[idx]
            dtw[:n, c] = (np.mod(w + PI, TWO_PI) - PI).astype(np.float32)
    for c in range(NCH):
        n = CHUNK_REAL[c]
        idx = perm[c, :n]
        waT[:, c * P:c * P + n] = W_amp[idx].T

    kT = np.zeros((ncp * P, ncp * P), dtype=np.float32)
    for jc in range(ncp):
        nj = CHUNK_REAL[jc]
        jdx = perm[jc, :nj]
        for ic in range(ncp):
            ni = CHUNK_REAL[ic]
            idx = perm[ic, :ni]
            kT[jc * P:jc * P + nj, ic * P:ic * P + ni] = \
                (float(STRIDE) if fast_rot else 1.0) * DT * \
                K[np.ix_(idx, jdx)].T

    nz = [
        (jc, ic)
        for jc in range(ncp)
        for ic in range(ncp)
        if np.any(kT[jc * P:(jc + 1) * P, ic * P:(ic + 1) * P] != 0.0)
    ]
    if (0, 0) not in nz:
        nz.append((0, 0))     # carries the band-sum indicator columns
    for ic in range(1, ncp):
        if not any(i2 == ic for (_, i2) in nz):
            nz.append((ic, ic))
    nz_pairs = tuple(sorted(nz))

    # fuse delta/theta indicator columns into the (0,0) block pads:
    # cols 96,97 tap the sin half (phi_pad=0), 98,99 the cos half (pi/2)
    for cc in (96, 98):
        kT[0:N_DELTA, cc] = 1.0
        kT[N_DELTA:96, cc + 1] = 1.0

    key = (nz_pairs, fast_rot, has_res, ncp)
    if key not in _COMPILED:
        _COMPILED[key] = _build_program(nz_pairs, fast_rot, has_res, ncp)
    nc = _COMPILED[key]

    # kt blocks packed [128, (jc*ncp+ic)*128 .. +128]
    ktp = np.zeros((P, ncp * ncp * P), dtype=ml_dtypes.bfloat16)
    for jc in range(ncp):
        for ic in range(ncp):
            ktp[:, (jc * ncp + ic) * P:(jc * ncp + ic + 1) * P] = \
                kT[jc * P:(jc + 1) * P, ic * P:(ic + 1) * P]
    wpp = chunk_pack(wpT).astype(ml_dtypes.bfloat16)
    wap = chunk_pack(waT).astype(ml_dtypes.bfloat16)
    padphi = np.zeros((4, max(BHS)), dtype=ml_dtypes.bfloat16)
    padphi[2:4, :] = np.float32(PI / 2.0)
    in_maps = []
    for i in range(N_CORES):
        xs = x[i * BL:(i + 1) * BL]
        xsp = chunk_pack(np.ascontiguousarray(xs.T))
        in_maps.append({
            "xbT": xsp.astype(ml_dtypes.bfloat16),
            "wpT": wpp, "waT": wap, "ktT": ktp, "dtw": dtw,
            "padphi": padphi,
        })

    res_run = run_bass_kernel_spmd(nc, in_maps, core_ids=list(range(N_CORES)))

    # ---- host-side unshard + exact amp reconstruction (f64) ----
    out = np.empty((BATCH, N_TOTAL), dtype=np.float32)
    NDUMP = (N_STEPS // STRIDE + 1) if fast_rot else N_STEPS
    kk = np.arange(1, N_STEPS + 1)
    dmap = (kk // STRIDE) if fast_rot else (kk - 1)
    ks = kk.astype(np.float64)
    if fast_rot:
        rotd = ks * A_band[0]
        rott = ks * A_band[1]
    else:
        rotd = np.zeros(N_STEPS)
        rott = np.zeros(N_STEPS)

    for i in range(N_CORES):
        r = res_run.results[i]
        amp0v = np.maximum(np.abs(r["amp0"].astype(np.float64)), EPS)
        bsv = r["bsums"].astype(np.float64)      # [4, NDUMP*BL]
        if not fast_rot:
            bsv = -bsv                           # fallback d = -coup sign
        f = np.empty((BL, N_STEPS, 2))
        off = 0
        for h in range(NH):
            bh = BHS[h]
            blk = bsv[:, off:off + NDUMP * bh].reshape(4, NDUMP, bh)
            blk = blk[:, dmap]                    # expand to N_STEPS
            S = blk[0:2]                          # [2(d,t), k, j] sin sums
            C = -blk[2:4]
            R = np.sqrt(S * S + C * C)
            R = np.maximum(R, 1e-30)
            cd = (C[0] * np.cos(rotd)[:, None]
                  - S[0] * np.sin(rotd)[:, None]) / R[0]
            ct = (C[1] * np.cos(rott)[:, None]
                  - S[1] * np.sin(rott)[:, None]) / R[1]
            sl = slice(OFFS[h], OFFS[h] + bh)
            f[sl, :, 0] = 1.0 + DT * PAC * cd.T   # theta-band factor
            f[sl, :, 1] = 1.0 + DT * PAC * ct.T   # gamma-band factor
            off += NDUMP * bh
        Pk = np.cumprod(f, axis=1)
        m = np.minimum.accumulate(Pk, axis=1)
        Pn = Pk[:, -1]
        mn = m[:, -1]
        Pfac = np.ones((BL, 3))
        Efac = np.ones((BL, 3))
        Pfac[:, 1:] = Pn
        Efac[:, 1:] = Pn / mn
        a0 = np.empty((BL, N_TOTAL))
        ao = 0
        for h in range(NH):
            bh = BHS[h]
            for c in range(NCH):
                n = CHUNK_REAL[c]
                idx = perm[c, :n]
                a0[OFFS[h]:OFFS[h] + bh, idx] = \
                    amp0v[:n, ao + c * bh:ao + (c + 1) * bh].T
            ao += NCH * bh
        amp = np.maximum(a0 * Pfac[:, band_of], EPS * Efac[:, band_of])
        out[i * BL:(i + 1) * BL] = amp.astype(np.float32)
    return out


# revision 26
# speedup vs baseline: 1.8920x; 1.0516x over previous
"""Trainium2 Bass kernel for DiscreteDeltaThetaGammaLayer.

Coupled Kuramoto-oscillator recurrence:
  phase0 = (x @ W_phase.T) mod 2pi ; amp0 = max(|x @ W_amp.T|, eps)
  32 steps of: intra-band Kuramoto coupling (phase), PAC amplitude modulation
  output: final amp  (4096, 352) f32

Key structural facts exploited (checked on the host, with a full-width
fallback if they don't hold):
  - The output uses only amp0 and the delta/theta band MEAN phases (PAC);
    gamma phases never feed them when K[delta+theta, gamma] == 0 (block-diag
    K), so the phase recurrence runs on the 96 delta+theta oscillators only.
  - Rotating frame per band (phi~ = phi - k*dt*omega_band) removes the
    per-step omega add and the wrap; the Sin LUT is accurate to |x|<~pi+0.65
    and coupling drift is bounded by 32*dt*max|K|_row <= 0.64. The host
    de-rotates the stashed band sums exactly in f64.
  - The band means drift only O(1e-4) under the weak coupling, so the
    coupling is integrated with STRIDE reference-steps per device iteration
    (frozen coupling field), with band sums stashed each iteration and
    host-side nearest-dump expansion. Measured output error ~2e-3 incl. the
    bf16 amp path (tolerance 2e-2).
  - Band sums ride free in the coupling matmul: the K chunk-0 block's pad
    lhsT columns 96:99 carry delta/theta indicators against pinned pad
    phases (0, 0, pi/2, pi/2), so d = mm1-mm2 holds (Sd, St, -Cd, -Ct) on
    partitions 96:100; the stash is a tiny SBUF->DRAM DMA.
  - amp path reuses the bf16 x and bf16 W_amp (error ~2e-3 on output);
    host reconstructs the clamped amp recurrence in closed form (exact).
"""

import math
import sys

sys.path.insert(0, "/opt/trn_rl_repo")

import numpy as np

# ---- problem constants (module hyperparameters) ----
N_DELTA, N_THETA, N_GAMMA = 32, 64, 256
N_TOTAL = 352
N_DIMS = 1024
BATCH = 4096
N_STEPS = 32
DT = 0.01
PAC = 0.3
EPS = 1e-6
TWO_PI = 2.0 * math.pi
PI = math.pi

N_CORES = 8
BL = BATCH // N_CORES          # 512 batch rows per core
BHS = [256, 256]               # independent streams (latency hiding)
OFFS = [0, 256]
NH = len(BHS)
P = 128
NCH = 3                        # oscillator chunks for the amp path
CHUNK_REAL = [96, 128, 128]
KD = N_DIMS // P               # 8 contraction chunks for the projections

LAST_EXEC_NS = None
_COMPILED = {}
_WRAP_SUB = None

# drift budget: |phi~| may reach pi + DRIFT_MAX with Sin LUT err ~1.2e-3
DRIFT_MAX = 0.66
STRIDE = 32                    # reference steps per device iteration


def _osc_perm():
    """orig oscillator index for each (chunk, partition); -1 for pads."""
    perm = -np.ones((NCH, P), dtype=np.int64)
    perm[0, :96] = np.arange(96)           # delta + theta
    perm[1, :] = 96 + np.arange(128)       # gamma 0:128
    perm[2, :] = 224 + np.arange(128)      # gamma 128:256
    return perm


def _get_wrap_sub():
    """Custom DVE op: out = wrap((in0 - in1) + s0) into [-s1, s1], period imm2."""
    global _WRAP_SUB
    if _WRAP_SUB is not None:
        return _WRAP_SUB
    from concourse.dve_spec import C0, C1, C2, Spec, Src0, Src1, lower
    from concourse.dve_uop import DveOpSpec
    import concourse.dve_ops as dvo

    def _ref(in0, in1, s0, s1, imm2):
        y = (in0 - in1) + s0
        return (y + imm2 * ((y < -s1).astype(np.float32)
                            - (y > s1).astype(np.float32))).astype(np.float32)

    _y = (Src0 - Src1) + C0
    spec = Spec(body=_y + C2 * ((_y < -C1) - (_y > C1)), reference=_ref)
    shas = {}
    for ver in ("v3", "v4"):
        tmp = DveOpSpec(name="WRAP_SUB_KERNEL", opcode=31,
                        uops=lower(spec, ver=ver), rd1_en=True)
        shas[ver] = tmp.sha(ver)
    op = dvo.DveOp("WRAP_SUB_KERNEL", spec, subdim=False, uops_sha=shas)
    dvo.OPS.append(op)
    dvo.CUSTOM_DVE_SPECS[op.name] = op.spec
    dvo._SUB_OPCODE_FOR_NAME[op.name] = dvo._CUSTOM_DVE_ROW_BASE + len(dvo.OPS) - 1
    _WRAP_SUB = op
    return op


def _build_program(nz_pairs, fast_rot, has_res, ncp):
    """ncp: number of phase chunks (1 when gamma is output-irrelevant, else 3).
    fast_rot: rotating frame + stride-STRIDE coupling, no wrap.
    Fallback: per-step wrap with dt*omega in s0, stride 1."""
    import concourse.bass as bass
    import concourse.tile as tile
    from concourse import bacc, mybir

    f32 = mybir.dt.float32
    bf16 = mybir.dt.bfloat16
    u16 = mybir.dt.uint16
    AF = mybir.ActivationFunctionType
    ALU = mybir.AluOpType

    wrap_sub = _get_wrap_sub() if not fast_rot else None

    nc = bacc.Bacc("TRN2", target_bir_lowering=False, debug=False)

    # ---- DRAM I/O (host pre-packs k-chunks along the free dim) ----
    xbT = nc.dram_tensor("xbT", [P, KD * BL], bf16, kind="ExternalInput").ap()
    wpT = nc.dram_tensor("wpT", [P, KD * ncp * P], bf16,
                         kind="ExternalInput").ap()
    waT = nc.dram_tensor("waT", [P, KD * NCH * P], bf16,
                         kind="ExternalInput").ap()
    # constants blob: kt blocks | dtw (as bf16-pair cols kept f32 separate) |
    # padphi rows. Layout: [P, ncp*ncp*P (bf16 kt) + max(BHS) (bf16 padphi)]
    # and dtw as its own small f32 tensor (loaded only when needed).
    KTW = ncp * ncp * P
    PPW = max(BHS)
    constT = nc.dram_tensor("constT", [P, KTW + PPW], bf16,
                            kind="ExternalInput").ap()
    dtw = nc.dram_tensor("dtw", [P, ncp], f32, kind="ExternalInput").ap()

    amp0_out = nc.dram_tensor("amp0", [P, NCH * BL], f32,
                              kind="ExternalOutput").ap()

    MS = (N_STEPS // STRIDE + 1) if fast_rot else (N_STEPS + 1)
    NDUMP = MS if fast_rot else N_STEPS
    # stash: rows (Sd, St, -Cd, -Ct); per stream block of NDUMP*bh cols
    bs_out = nc.dram_tensor("bsums", [4, NDUMP * BL], bf16,
                            kind="ExternalOutput").ap()

    with tile.TileContext(nc) as tc:
        with (
            tc.tile_pool(name="state", bufs=1) as state_pool,
            tc.tile_pool(name="weights", bufs=1) as wpool,
            tc.tile_pool(name="work", bufs=2) as work,
            tc.tile_pool(name="psum", bufs=1, space="PSUM") as psum,
        ):
            # ---- constants; warm the Sin table during the loads ----
            pihalf = wpool.tile([P, 1], f32, tag="pihalf", name="pihalf")
            nc.vector.memset(pihalf[:], PI / 2.0)
            warm = wpool.tile([P, 1], bf16, tag="warm", name="warm")
            nc.scalar.activation(warm[:], pihalf[:], AF.Sin)

            # ---- phase-path loads first: they gate the recurrence ----
            xall = wpool.tile([P, KD * BL], bf16, tag="xall", name="xall")
            wall = wpool.tile([P, KD * ncp * P], bf16, tag="wall", name="wall")
            call = wpool.tile([P, KTW + PPW], bf16, tag="call", name="call")
            waall = wpool.tile([P, KD * NCH * P], bf16, tag="waall",
                               name="waall")
            hx = KD * BL // 2
            hw = KD * NCH * P // 2
            nc.sync.dma_start(xall[:, 0:hx], xbT[:, 0:hx])
            nc.scalar.dma_start(wall[:], wpT[:])
            nc.scalar.dma_start(call[:], constT[:])
            nc.sync.dma_start(xall[:, hx:], xbT[:, hx:])
            nc.scalar.dma_start(waall[:, 0:hw], waT[:, 0:hw])
            nc.sync.dma_start(waall[:, hw:], waT[:, hw:])
            xk = [xall[:, k * BL:(k + 1) * BL] for k in range(KD)]
            wk = [wall[:, k * ncp * P:(k + 1) * ncp * P] for k in range(KD)]
            wak = [waall[:, k * NCH * P:(k + 1) * NCH * P] for k in range(KD)]
            kt_sb = {}
            for (jc, ic) in nz_pairs:
                o = (jc * ncp + ic) * P
                kt_sb[(jc, ic)] = call[:, o:o + P]
            padphi_sb = call[:, KTW:KTW + PPW]
            dtw_sb = None
            if (not fast_rot) or has_res:
                dtw_sb = wpool.tile([P, ncp], f32, tag="dtw", name="dtw_sb")
                nc.scalar.dma_start(dtw_sb[:], dtw[:])

            # ---- per-stream state (phase width = ncp*bh) ----
            boff = [NDUMP * sum(BHS[:h]) for h in range(NH)]
            phi, cs, mmt, dts, pabs, vu = [], [], [], [], [], []
            for h in range(NH):
                bh = BHS[h]
                wh = ncp * bh
                phi.append(state_pool.tile([P, wh], bf16, tag=f"phi{h}",
                                           name=f"phi{h}"))
                cs.append(state_pool.tile([P, 2 * wh], bf16, tag=f"cs{h}",
                                          name=f"cs{h}"))
                mmt.append(state_pool.tile([P, 2 * wh], bf16, tag=f"mm{h}",
                                           name=f"mm{h}"))
                dts.append([state_pool.tile([P, wh], bf16, tag=f"d{h}_{pb}",
                                            name=f"d{h}_{pb}")
                            for pb in range(2)])
                pabs.append(work.tile([P, wh], bf16, tag=f"pabs{h}",
                                      name=f"pabs{h}"))
                vu.append(psum.tile([P, 2 * wh], f32, tag=f"vu{h}",
                                    name=f"vu{h}"))
                # tap partitions: 96,97 phi=0 (cos=1,sin=0); 98,99 pi/2;
                # 100:128 zeroed. wrap later writes partitions 0:96 only.
                nc.vector.memset(phi[h][96:128, :], 0.0)
                nc.vector.tensor_copy(phi[h][96:100, 0:bh],
                                      padphi_sb[96:100, 0:bh])
            amp_acc = psum.tile([P, NCH * BL], f32, tag="ampacc",
                                name="amp_acc")

            # ---- phase projections -> phi (per stream) ----
            for h in range(NH):
                bh = BHS[h]
                wh = ncp * bh
                for c in range(ncp):
                    acc = vu[h][:, c * bh:(c + 1) * bh]
                    for k in range(KD):
                        nc.tensor.matmul(
                            acc, wk[k][:, c * P:(c + 1) * P],
                            xk[k][:, OFFS[h]:OFFS[h] + bh],
                            start=(k == 0), stop=(k == KD - 1),
                        )
                nc.vector.add_range_wrap(phi[h][0:96, 0:bh],
                                         vu[h][0:96, 0:bh], 0.0, PI, TWO_PI)
                if ncp > 1:
                    nc.vector.add_range_wrap(phi[h][:, bh:wh],
                                             vu[h][:, bh:wh], 0.0, PI, TWO_PI)

            # ---- amp path: one 512-wide pass, per-chunk abs + DMA ----
            ab = work.tile([P, NCH * BL], f32, tag="abs0", name="ab")

            def emit_amp_path():
                for c in range(NCH):
                    acc = amp_acc[:, c * BL:(c + 1) * BL]
                    for k in range(KD):
                        nc.tensor.matmul(
                            acc, wak[k][:, c * P:(c + 1) * P], xk[k],
                            start=(k == 0), stop=(k == KD - 1),
                        )
                    nc.scalar.activation(ab[:, c * BL:(c + 1) * BL],
                                         acc, AF.Abs)
                    nc.scalar.dma_start(
                        amp0_out[:, c * BL:(c + 1) * BL],
                        ab[:, c * BL:(c + 1) * BL])

            # ---- the recurrence ----
            amp_at = min(1, MS - 1)
            for it in range(MS):
                if it == amp_at:
                    emit_amp_path()
                for h in range(NH):
                    bh = BHS[h]
                    wh = ncp * bh
                    ph = phi[h]
                    sin = cs[h][:, wh:2 * wh]
                    cos = cs[h][:, 0:wh]
                    last = (it == MS - 1)
                    nc.scalar.activation(sin[:], ph[:], AF.Sin)
                    nc.vector.tensor_scalar(
                        pabs[h][:].bitcast(u16), ph[:].bitcast(u16),
                        0x7FFF, None, ALU.bitwise_and)
                    nc.scalar.activation(cos[:], pabs[h][:], AF.Sin,
                                         bias=pihalf[:], scale=-1.0)

                    # coupling: [v | u] = (S*dt*K) [sin | cos]; chunk-0 block
                    # also emits band sums on partitions 96:100
                    for ic in range(ncp):
                        jcs = [jc for (jc, i2) in nz_pairs if i2 == ic]
                        for half, srcoff in ((0, wh), (1, 0)):
                            dst = vu[h][:, half * wh + ic * bh:
                                        half * wh + (ic + 1) * bh]
                            for n, jc in enumerate(jcs):
                                src = cs[h][:, srcoff + jc * bh:
                                            srcoff + (jc + 1) * bh]
                                nc.tensor.matmul(
                                    dst, kt_sb[(jc, ic)], src,
                                    start=(n == 0), stop=(n == len(jcs) - 1),
                                )

                    # mm = [cos|sin] * [v|u]; d = c*v - s*u (fast) or -coup
                    # (fallback, for WRAP_SUB's wrap((phi - d) + s0)).
                    # d partitions 96:100 hold (Sd, St, -Cd, -Ct).
                    dtile = dts[h][it % 2]
                    nc.vector.tensor_tensor(mmt[h][:], cs[h][:], vu[h][:],
                                            ALU.mult)
                    a, b = (0, wh) if fast_rot else (wh, 0)
                    nc.vector.tensor_tensor(
                        dtile[:], mmt[h][:, a:a + wh],
                        mmt[h][:, b:b + wh], ALU.subtract)
                    if fast_rot or it > 0:
                        slot = it if fast_rot else it - 1
                        so = boff[h] + slot * bh
                        nc.sync.dma_start(bs_out[:, so:so + bh],
                                          dtile[96:100, 0:bh])
                    if last:
                        continue

                    # phi update (tap partitions 96:100 excluded on chunk 0)
                    if fast_rot:
                        if has_res:
                            for c in range(ncp):
                                pe = 96 if c == 0 else P
                                nc.vector.scalar_tensor_tensor(
                                    ph[0:pe, c * bh:(c + 1) * bh],
                                    dtile[0:pe, c * bh:(c + 1) * bh],
                                    dtw_sb[0:pe, c:c + 1],
                                    ph[0:pe, c * bh:(c + 1) * bh],
                                    ALU.add, ALU.add)
                        else:
                            nc.vector.tensor_tensor(
                                ph[0:96, 0:bh], ph[0:96, 0:bh],
                                dtile[0:96, 0:bh], ALU.add)
                            if ncp > 1:
                                nc.vector.tensor_tensor(
                                    ph[:, bh:wh], ph[:, bh:wh],
                                    dtile[:, bh:wh], ALU.add)
                    else:
                        for c in range(ncp):
                            pe = 96 if c == 0 else P
                            nc.vector._custom_dve(
                                wrap_sub,
                                out=ph[0:pe, c * bh:(c + 1) * bh],
                                in0=ph[0:pe, c * bh:(c + 1) * bh],
                                in1=dtile[0:pe, c * bh:(c + 1) * bh],
                                s0=dtw_sb[0:pe, c:c + 1],
                                s1=PI,
                                imm2=TWO_PI,
                            )

    nc.compile()
    return nc


def kernel(x, W_phase, W_amp, omega, K):
    import ml_dtypes
    from concourse.bass_utils import run_bass_kernel_spmd

    x = np.asarray(x, dtype=np.float32)
    W_phase = np.asarray(W_phase, dtype=np.float32)
    W_amp = np.asarray(W_amp, dtype=np.float32)
    omega = np.asarray(omega, dtype=np.float32)
    K = np.asarray(K, dtype=np.float32)

    perm = _osc_perm()
    band_of = np.zeros(N_TOTAL, dtype=np.int64)
    band_of[N_DELTA:N_DELTA + N_THETA] = 1
    band_of[N_DELTA + N_THETA:] = 2

    # ---- structural checks ----
    Kf = K.astype(np.float64)
    dtww = DT * omega.astype(np.float64)
    A_band = np.array([dtww[band_of == b].mean() for b in range(3)])
    res = dtww - A_band[band_of]
    row_l1 = DT * np.abs(Kf).sum(axis=1)
    drift = N_STEPS * (np.abs(res) + row_l1).max()
    ii, jj = np.nonzero(K)
    frames_ok = np.allclose(A_band[band_of[ii]], A_band[band_of[jj]],
                            rtol=0, atol=1e-12) if len(ii) else True
    fast_rot = bool(frames_ok and drift <= DRIFT_MAX)
    has_res = bool(fast_rot and np.abs(res).max() > 1e-12)
    # gamma is output-irrelevant iff it never couples into delta/theta
    g_isolated = not np.any(Kf[0:96, 96:] != 0.0)
    ncp = 1 if g_isolated else NCH

    # ---- host-side packing ----
    def chunk_pack(a):
        # [N_DIMS, C] -> [128, KD*C] with k-chunks along free dim
        C = a.shape[1]
        return np.ascontiguousarray(
            a.reshape(KD, P, C).transpose(1, 0, 2).reshape(P, KD * C))

    wpT = np.zeros((N_DIMS, ncp * P), dtype=np.float32)
    waT = np.zeros((N_DIMS, NCH * P), dtype=np.float32)
    dtw = np.zeros((P, ncp), dtype=np.float32)
    for c in range(ncp):
        n = CHUNK_REAL[c]
        idx = perm[c, :n]
        wpT[:, c * P:c * P + n] = W_phase[idx].T
        if fast_rot:
            dtw[:n, c] = float(STRIDE) * res[idx].astype(np.float32)
        else:
            w = dtww[idx]
            dtw[:n, c] = (np.mod(w + PI, TWO_PI) - PI).astype(np.float32)
    for c in range(NCH):
        n = CHUNK_REAL[c]
        idx = perm[c, :n]
        waT[:, c * P:c * P + n] = W_amp[idx].T

    kT = np.zeros((ncp * P, ncp * P), dtype=np.float32)
    for jc in range(ncp):
        nj = CHUNK_REAL[jc]
        jdx = perm[jc, :nj]
        for ic in range(ncp):
            ni = CHUNK_REAL[ic]
            idx = perm[ic, :ni]
            kT[jc * P:jc * P + nj, ic * P:ic * P + ni] = \
                (float(STRIDE) if fast_rot else 1.0) * DT * \
                K[np.ix_(idx, jdx)].T

    nz = [
        (jc, ic)
        for jc in range(ncp)
        for ic in range(ncp)
        if np.any(kT[jc * P:(jc + 1) * P, ic * P:(ic + 1) * P] != 0.0)
    ]
    if (0, 0) not in nz:
        nz.append((0, 0))     # carries the band-sum indicator columns
    for ic in range(1, ncp):
        if not any(i2 == ic for (_, i2) in nz):
            nz.append((ic, ic))
    nz_pairs = tuple(sorted(nz))

    # fuse delta/theta indicator columns into the (0,0) block pads:
    # cols 96,97 tap the sin half (phi_pad=0), 98,99 the cos half (pi/2)
    for cc in (96, 98):
        kT[0:N_DELTA, cc] = 1.0
        kT[N_DELTA:96, cc + 1] = 1.0

    key = (nz_pairs, fast_rot, has_res, ncp)
    if key not in _COMPILED:
        _COMPILED[key] = _build_program(nz_pairs, fast_rot, has_res, ncp)
    nc = _COMPILED[key]

    # const blob: kt blocks + padphi rows (on partitions 96:100)
    KTW = ncp * ncp * P
    PPW = max(BHS)
    constp = np.zeros((P, KTW + PPW), dtype=ml_dtypes.bfloat16)
    for jc in range(ncp):
        for ic in range(ncp):
            constp[:, (jc * ncp + ic) * P:(jc * ncp + ic + 1) * P] = \
                kT[jc * P:(jc + 1) * P, ic * P:(ic + 1) * P]
    constp[98:100, KTW:] = np.float32(PI / 2.0)
    wpp = chunk_pack(wpT).astype(ml_dtypes.bfloat16)
    wap = chunk_pack(waT).astype(ml_dtypes.bfloat16)
    in_maps = []
    for i in range(N_CORES):
        xs = x[i * BL:(i + 1) * BL]
        xsp = chunk_pack(np.ascontiguousarray(xs.T))
        in_maps.append({
            "xbT": xsp.astype(ml_dtypes.bfloat16),
            "wpT": wpp, "waT": wap, "constT": constp, "dtw": dtw,
        })

    res_run = run_bass_kernel_spmd(nc, in_maps, core_ids=list(range(N_CORES)))

    # ---- host-side unshard + exact amp reconstruction (f64) ----
    out = np.empty((BATCH, N_TOTAL), dtype=np.float32)
    NDUMP = (N_STEPS // STRIDE + 1) if fast_rot else N_STEPS
    kk = np.arange(1, N_STEPS + 1)
    dmap = (kk // STRIDE) if fast_rot else (kk - 1)
    ks = kk.astype(np.float64)
    if fast_rot:
        rotd = ks * A_band[0]
        rott = ks * A_band[1]
    else:
        rotd = np.zeros(N_STEPS)
        rott = np.zeros(N_STEPS)

    for i in range(N_CORES):
        r = res_run.results[i]
        amp0v = np.maximum(np.abs(r["amp0"].astype(np.float64)), EPS)
        bsv = r["bsums"].astype(np.float64)      # [4, NDUMP*BL]
        if not fast_rot:
            bsv = -bsv                           # fallback d = -coup sign
        f = np.empty((BL, N_STEPS, 2))
        off = 0
        for h in range(NH):
            bh = BHS[h]
            blk = bsv[:, off:off + NDUMP * bh].reshape(4, NDUMP, bh)
            blk = blk[:, dmap]                    # expand to N_STEPS
            S = blk[0:2]                          # [2(d,t), k, j] sin sums
            C = -blk[2:4]
            R = np.sqrt(S * S + C * C)
            R = np.maximum(R, 1e-30)
            cd = (C[0] * np.cos(rotd)[:, None]
                  - S[0] * np.sin(rotd)[:, None]) / R[0]
            ct = (C[1] * np.cos(rott)[:, None]
                  - S[1] * np.sin(rott)[:, None]) / R[1]
            sl = slice(OFFS[h], OFFS[h] + bh)
            f[sl, :, 0] = 1.0 + DT * PAC * cd.T   # theta-band factor
            f[sl, :, 1] = 1.0 + DT * PAC * ct.T   # gamma-band factor
            off += NDUMP * bh
        Pk = np.cumprod(f, axis=1)
        m = np.minimum.accumulate(Pk, axis=1)
        Pn = Pk[:, -1]
        mn = m[:, -1]
        Pfac = np.ones((BL, 3))
        Efac = np.ones((BL, 3))
        Pfac[:, 1:] = Pn
        Efac[:, 1:] = Pn / mn
        a0 = np.empty((BL, N_TOTAL))
        for c in range(NCH):
            n = CHUNK_REAL[c]
            idx = perm[c, :n]
            a0[:, idx] = amp0v[:n, c * BL:(c + 1) * BL].T
        amp = np.maximum(a0 * Pfac[:, band_of], EPS * Efac[:, band_of])
        out[i * BL:(i + 1) * BL] = amp.astype(np.float32)
    return out


# revision 27
# speedup vs baseline: 2.0581x; 1.0878x over previous
"""Trainium2 Bass kernel for DiscreteDeltaThetaGammaLayer.

Coupled Kuramoto-oscillator recurrence:
  phase0 = (x @ W_phase.T) mod 2pi ; amp0 = max(|x @ W_amp.T|, eps)
  32 steps of: intra-band Kuramoto coupling (phase), PAC amplitude modulation
  output: final amp  (4096, 352) f32

Key structural facts exploited (checked on the host, with a full-width
fallback if they don't hold):
  - The output uses only amp0 and the delta/theta band MEAN phases (PAC);
    gamma phases never feed them when K[delta+theta, gamma] == 0 (block-diag
    K), so the phase recurrence runs on the 96 delta+theta oscillators only.
  - Rotating frame per band (phi~ = phi - k*dt*omega_band) removes the
    per-step omega add and the wrap; the Sin LUT is accurate to |x|<~pi+0.65
    and coupling drift is bounded by 32*dt*max|K|_row <= 0.64. The host
    de-rotates the stashed band sums exactly in f64.
  - The band means drift only O(1e-4) under the weak coupling, so the
    coupling is integrated with STRIDE reference-steps per device iteration
    (frozen coupling field), with band sums stashed each iteration and
    host-side nearest-dump expansion. Measured output error ~2e-3 incl. the
    bf16 amp path (tolerance 2e-2).
  - Band sums ride free in the coupling matmul: the K chunk-0 block's pad
    lhsT columns 96:99 carry delta/theta indicators against pinned pad
    phases (0, 0, pi/2, pi/2), so d = mm1-mm2 holds (Sd, St, -Cd, -Ct) on
    partitions 96:100; the stash is a tiny SBUF->DRAM DMA.
  - amp path reuses the bf16 x and bf16 W_amp (error ~2e-3 on output);
    host reconstructs the clamped amp recurrence in closed form (exact).
"""

import math
import sys

sys.path.insert(0, "/opt/trn_rl_repo")

import numpy as np

# ---- problem constants (module hyperparameters) ----
N_DELTA, N_THETA, N_GAMMA = 32, 64, 256
N_TOTAL = 352
N_DIMS = 1024
BATCH = 4096
N_STEPS = 32
DT = 0.01
PAC = 0.3
EPS = 1e-6
TWO_PI = 2.0 * math.pi
PI = math.pi

N_CORES = 8
BL = BATCH // N_CORES          # 512 batch rows per core
BHS = [256, 256]               # independent streams (latency hiding)
OFFS = [0, 256]
NH = len(BHS)
P = 128
NCH = 3                        # oscillator chunks for the amp path
CHUNK_REAL = [96, 128, 128]
KD = N_DIMS // P               # 8 contraction chunks for the projections

LAST_EXEC_NS = None
_COMPILED = {}
_WRAP_SUB = None

# drift budget: |phi~| may reach pi + DRIFT_MAX with Sin LUT err ~1.2e-3
DRIFT_MAX = 0.66
STRIDE = 32                    # reference steps per device iteration


def _osc_perm():
    """orig oscillator index for each (chunk, partition); -1 for pads."""
    perm = -np.ones((NCH, P), dtype=np.int64)
    perm[0, :96] = np.arange(96)           # delta + theta
    perm[1, :] = 96 + np.arange(128)       # gamma 0:128
    perm[2, :] = 224 + np.arange(128)      # gamma 128:256
    return perm


def _get_wrap_sub():
    """Custom DVE op: out = wrap((in0 - in1) + s0) into [-s1, s1], period imm2."""
    global _WRAP_SUB
    if _WRAP_SUB is not None:
        return _WRAP_SUB
    from concourse.dve_spec import C0, C1, C2, Spec, Src0, Src1, lower
    from concourse.dve_uop import DveOpSpec
    import concourse.dve_ops as dvo

    def _ref(in0, in1, s0, s1, imm2):
        y = (in0 - in1) + s0
        return (y + imm2 * ((y < -s1).astype(np.float32)
                            - (y > s1).astype(np.float32))).astype(np.float32)

    _y = (Src0 - Src1) + C0
    spec = Spec(body=_y + C2 * ((_y < -C1) - (_y > C1)), reference=_ref)
    shas = {}
    for ver in ("v3", "v4"):
        tmp = DveOpSpec(name="WRAP_SUB_KERNEL", opcode=31,
                        uops=lower(spec, ver=ver), rd1_en=True)
        shas[ver] = tmp.sha(ver)
    op = dvo.DveOp("WRAP_SUB_KERNEL", spec, subdim=False, uops_sha=shas)
    dvo.OPS.append(op)
    dvo.CUSTOM_DVE_SPECS[op.name] = op.spec
    dvo._SUB_OPCODE_FOR_NAME[op.name] = dvo._CUSTOM_DVE_ROW_BASE + len(dvo.OPS) - 1
    _WRAP_SUB = op
    return op


def _build_program(nz_pairs, fast_rot, has_res, ncp):
    """ncp: number of phase chunks (1 when gamma is output-irrelevant, else 3).
    fast_rot: rotating frame + stride-STRIDE coupling, no wrap.
    Fallback: per-step wrap with dt*omega in s0, stride 1."""
    import concourse.bass as bass
    import concourse.tile as tile
    from concourse import bacc, mybir

    f32 = mybir.dt.float32
    bf16 = mybir.dt.bfloat16
    u16 = mybir.dt.uint16
    AF = mybir.ActivationFunctionType
    ALU = mybir.AluOpType

    wrap_sub = _get_wrap_sub() if not fast_rot else None

    nc = bacc.Bacc("TRN2", target_bir_lowering=False, debug=False)

    # ---- DRAM I/O (host pre-packs k-chunks along the free dim) ----
    xbT = nc.dram_tensor("xbT", [P, KD * BL], bf16, kind="ExternalInput").ap()
    wpT = nc.dram_tensor("wpT", [P, KD * ncp * P], bf16,
                         kind="ExternalInput").ap()
    waT = nc.dram_tensor("waT", [P, KD * NCH * P], bf16,
                         kind="ExternalInput").ap()
    # constants blob: kt blocks | dtw (as bf16-pair cols kept f32 separate) |
    # padphi rows. Layout: [P, ncp*ncp*P (bf16 kt) + max(BHS) (bf16 padphi)]
    # and dtw as its own small f32 tensor (loaded only when needed).
    KTW = ncp * ncp * P
    PPW = max(BHS)
    constT = nc.dram_tensor("constT", [P, KTW + PPW], bf16,
                            kind="ExternalInput").ap()
    dtw = nc.dram_tensor("dtw", [P, ncp], f32, kind="ExternalInput").ap()

    amp0_out = nc.dram_tensor("amp0", [P, NCH * BL], bf16,
                              kind="ExternalOutput").ap()

    MS = (N_STEPS // STRIDE + 1) if fast_rot else (N_STEPS + 1)
    NDUMP = MS if fast_rot else N_STEPS
    # stash: rows (Sd, St, -Cd, -Ct); per stream block of NDUMP*bh cols
    bs_out = nc.dram_tensor("bsums", [4, NDUMP * BL], bf16,
                            kind="ExternalOutput").ap()

    with tile.TileContext(nc) as tc:
        with (
            tc.tile_pool(name="state", bufs=1) as state_pool,
            tc.tile_pool(name="weights", bufs=1) as wpool,
            tc.tile_pool(name="work", bufs=2) as work,
            tc.tile_pool(name="psum", bufs=1, space="PSUM") as psum,
        ):
            # ---- constants; warm the Sin table during the loads ----
            pihalf = wpool.tile([P, 1], f32, tag="pihalf", name="pihalf")
            nc.vector.memset(pihalf[:], PI / 2.0)
            warm = wpool.tile([P, 1], bf16, tag="warm", name="warm")
            nc.scalar.activation(warm[:], pihalf[:], AF.Sin)

            # ---- phase-path loads first: they gate the recurrence ----
            xall = wpool.tile([P, KD * BL], bf16, tag="xall", name="xall")
            wall = wpool.tile([P, KD * ncp * P], bf16, tag="wall", name="wall")
            call = wpool.tile([P, KTW + PPW], bf16, tag="call", name="call")
            waall = wpool.tile([P, KD * NCH * P], bf16, tag="waall",
                               name="waall")
            qx = KD * BL // 4
            hw = KD * NCH * P // 2
            nc.sync.dma_start(xall[:, 0:qx], xbT[:, 0:qx])
            nc.scalar.dma_start(wall[:], wpT[:])
            nc.sync.dma_start(xall[:, qx:2 * qx], xbT[:, qx:2 * qx])
            nc.scalar.dma_start(call[:], constT[:])
            nc.sync.dma_start(xall[:, 2 * qx:3 * qx], xbT[:, 2 * qx:3 * qx])
            nc.scalar.dma_start(waall[:, 0:hw], waT[:, 0:hw])
            nc.sync.dma_start(xall[:, 3 * qx:], xbT[:, 3 * qx:])
            nc.scalar.dma_start(waall[:, hw:], waT[:, hw:])
            xk = [xall[:, k * BL:(k + 1) * BL] for k in range(KD)]
            wk = [wall[:, k * ncp * P:(k + 1) * ncp * P] for k in range(KD)]
            wak = [waall[:, k * NCH * P:(k + 1) * NCH * P] for k in range(KD)]
            kt_sb = {}
            for (jc, ic) in nz_pairs:
                o = (jc * ncp + ic) * P
                kt_sb[(jc, ic)] = call[:, o:o + P]
            padphi_sb = call[:, KTW:KTW + PPW]
            dtw_sb = None
            if (not fast_rot) or has_res:
                dtw_sb = wpool.tile([P, ncp], f32, tag="dtw", name="dtw_sb")
                nc.scalar.dma_start(dtw_sb[:], dtw[:])

            # ---- per-stream state (phase width = ncp*bh) ----
            boff = [NDUMP * sum(BHS[:h]) for h in range(NH)]
            phi, cs, mmt, dts, pabs, vu = [], [], [], [], [], []
            for h in range(NH):
                bh = BHS[h]
                wh = ncp * bh
                phi.append(state_pool.tile([P, wh], bf16, tag=f"phi{h}",
                                           name=f"phi{h}"))
                cs.append(state_pool.tile([P, 2 * wh], bf16, tag=f"cs{h}",
                                          name=f"cs{h}"))
                mmt.append(state_pool.tile([P, 2 * wh], bf16, tag=f"mm{h}",
                                           name=f"mm{h}"))
                dts.append([state_pool.tile([P, wh], bf16, tag=f"d{h}_{pb}",
                                            name=f"d{h}_{pb}")
                            for pb in range(2)])
                pabs.append(work.tile([P, wh], bf16, tag=f"pabs{h}",
                                      name=f"pabs{h}"))
                vu.append(psum.tile([P, 2 * wh], f32, tag=f"vu{h}",
                                    name=f"vu{h}"))
                # tap partitions: 96,97 phi=0 (cos=1,sin=0); 98,99 pi/2;
                # 100:128 zeroed. wrap later writes partitions 0:96 only.
                nc.vector.memset(phi[h][96:128, :], 0.0)
                nc.vector.tensor_copy(phi[h][96:100, 0:bh],
                                      padphi_sb[96:100, 0:bh])
            amp_acc = psum.tile([P, NCH * BL], f32, tag="ampacc",
                                name="amp_acc")

            # ---- phase projections -> phi (per stream) ----
            for h in range(NH):
                bh = BHS[h]
                wh = ncp * bh
                for c in range(ncp):
                    acc = vu[h][:, c * bh:(c + 1) * bh]
                    for k in range(KD):
                        nc.tensor.matmul(
                            acc, wk[k][:, c * P:(c + 1) * P],
                            xk[k][:, OFFS[h]:OFFS[h] + bh],
                            start=(k == 0), stop=(k == KD - 1),
                        )
                nc.vector.add_range_wrap(phi[h][0:96, 0:bh],
                                         vu[h][0:96, 0:bh], 0.0, PI, TWO_PI)
                if ncp > 1:
                    nc.vector.add_range_wrap(phi[h][:, bh:wh],
                                             vu[h][:, bh:wh], 0.0, PI, TWO_PI)

            # ---- amp path: one 512-wide pass, per-chunk abs + DMA ----
            ab = work.tile([P, NCH * BL], bf16, tag="abs0", name="ab")

            def emit_amp_path():
                for c in range(NCH):
                    acc = amp_acc[:, c * BL:(c + 1) * BL]
                    for k in range(KD):
                        nc.tensor.matmul(
                            acc, wak[k][:, c * P:(c + 1) * P], xk[k],
                            start=(k == 0), stop=(k == KD - 1),
                        )
                    nc.scalar.activation(ab[:, c * BL:(c + 1) * BL],
                                         acc, AF.Abs)
                    nc.scalar.dma_start(
                        amp0_out[:, c * BL:(c + 1) * BL],
                        ab[:, c * BL:(c + 1) * BL])

            # ---- the recurrence ----
            amp_at = min(1, MS - 1)
            for it in range(MS):
                if it == amp_at:
                    emit_amp_path()
                for h in range(NH):
                    bh = BHS[h]
                    wh = ncp * bh
                    ph = phi[h]
                    sin = cs[h][:, wh:2 * wh]
                    cos = cs[h][:, 0:wh]
                    last = (it == MS - 1)
                    nc.scalar.activation(sin[:], ph[:], AF.Sin)
                    nc.vector.tensor_scalar(
                        pabs[h][:].bitcast(u16), ph[:].bitcast(u16),
                        0x7FFF, None, ALU.bitwise_and)
                    nc.scalar.activation(cos[:], pabs[h][:], AF.Sin,
                                         bias=pihalf[:], scale=-1.0)

                    # coupling: [v | u] = (S*dt*K) [sin | cos]; chunk-0 block
                    # also emits band sums on partitions 96:100
                    for ic in range(ncp):
                        jcs = [jc for (jc, i2) in nz_pairs if i2 == ic]
                        for half, srcoff in ((0, wh), (1, 0)):
                            dst = vu[h][:, half * wh + ic * bh:
                                        half * wh + (ic + 1) * bh]
                            for n, jc in enumerate(jcs):
                                src = cs[h][:, srcoff + jc * bh:
                                            srcoff + (jc + 1) * bh]
                                nc.tensor.matmul(
                                    dst, kt_sb[(jc, ic)], src,
                                    start=(n == 0), stop=(n == len(jcs) - 1),
                                )

                    # mm = [cos|sin] * [v|u]; d = c*v - s*u (fast) or -coup
                    # (fallback, for WRAP_SUB's wrap((phi - d) + s0)).
                    # d partitions 96:100 hold (Sd, St, -Cd, -Ct).
                    dtile = dts[h][it % 2]
                    nc.vector.tensor_tensor(mmt[h][:], cs[h][:], vu[h][:],
                                            ALU.mult)
                    a, b = (0, wh) if fast_rot else (wh, 0)
                    nc.vector.tensor_tensor(
                        dtile[:], mmt[h][:, a:a + wh],
                        mmt[h][:, b:b + wh], ALU.subtract)
                    if fast_rot or it > 0:
                        slot = it if fast_rot else it - 1
                        so = boff[h] + slot * bh
                        nc.sync.dma_start(bs_out[:, so:so + bh],
                                          dtile[96:100, 0:bh])
                    if last:
                        continue

                    # phi update (tap partitions 96:100 excluded on chunk 0)
                    if fast_rot:
                        if has_res:
                            for c in range(ncp):
                                pe = 96 if c == 0 else P
                                nc.vector.scalar_tensor_tensor(
                                    ph[0:pe, c * bh:(c + 1) * bh],
                                    dtile[0:pe, c * bh:(c + 1) * bh],
                                    dtw_sb[0:pe, c:c + 1],
                                    ph[0:pe, c * bh:(c + 1) * bh],
                                    ALU.add, ALU.add)
                        else:
                            nc.vector.tensor_tensor(
                                ph[0:96, 0:bh], ph[0:96, 0:bh],
                                dtile[0:96, 0:bh], ALU.add)
                            if ncp > 1:
                                nc.vector.tensor_tensor(
                                    ph[:, bh:wh], ph[:, bh:wh],
                                    dtile[:, bh:wh], ALU.add)
                    else:
                        for c in range(ncp):
                            pe = 96 if c == 0 else P
                            nc.vector._custom_dve(
                                wrap_sub,
                                out=ph[0:pe, c * bh:(c + 1) * bh],
                                in0=ph[0:pe, c * bh:(c + 1) * bh],
                                in1=dtile[0:pe, c * bh:(c + 1) * bh],
                                s0=dtw_sb[0:pe, c:c + 1],
                                s1=PI,
                                imm2=TWO_PI,
                            )

    nc.compile()
    return nc


def kernel(x, W_phase, W_amp, omega, K):
    import ml_dtypes
    from concourse.bass_utils import run_bass_kernel_spmd

    x = np.asarray(x, dtype=np.float32)
    W_phase = np.asarray(W_phase, dtype=np.float32)
    W_amp = np.asarray(W_amp, dtype=np.float32)
    omega = np.asarray(omega, dtype=np.float32)
    K = np.asarray(K, dtype=np.float32)

    perm = _osc_perm()
    band_of = np.zeros(N_TOTAL, dtype=np.int64)
    band_of[N_DELTA:N_DELTA + N_THETA] = 1
    band_of[N_DELTA + N_THETA:] = 2

    # ---- structural checks ----
    Kf = K.astype(np.float64)
    dtww = DT * omega.astype(np.float64)
    A_band = np.array([dtww[band_of == b].mean() for b in range(3)])
    res = dtww - A_band[band_of]
    row_l1 = DT * np.abs(Kf).sum(axis=1)
    drift = N_STEPS * (np.abs(res) + row_l1).max()
    ii, jj = np.nonzero(K)
    frames_ok = np.allclose(A_band[band_of[ii]], A_band[band_of[jj]],
                            rtol=0, atol=1e-12) if len(ii) else True
    fast_rot = bool(frames_ok and drift <= DRIFT_MAX)
    has_res = bool(fast_rot and np.abs(res).max() > 1e-12)
    # gamma is output-irrelevant iff it never couples into delta/theta
    g_isolated = not np.any(Kf[0:96, 96:] != 0.0)
    ncp = 1 if g_isolated else NCH

    # ---- host-side packing ----
    def chunk_pack(a):
        # [N_DIMS, C] -> [128, KD*C] with k-chunks along free dim
        C = a.shape[1]
        return np.ascontiguousarray(
            a.reshape(KD, P, C).transpose(1, 0, 2).reshape(P, KD * C))

    wpT = np.zeros((N_DIMS, ncp * P), dtype=np.float32)
    waT = np.zeros((N_DIMS, NCH * P), dtype=np.float32)
    dtw = np.zeros((P, ncp), dtype=np.float32)
    for c in range(ncp):
        n = CHUNK_REAL[c]
        idx = perm[c, :n]
        wpT[:, c * P:c * P + n] = W_phase[idx].T
        if fast_rot:
            dtw[:n, c] = float(STRIDE) * res[idx].astype(np.float32)
        else:
            w = dtww[idx]
            dtw[:n, c] = (np.mod(w + PI, TWO_PI) - PI).astype(np.float32)
    for c in range(NCH):
        n = CHUNK_REAL[c]
        idx = perm[c, :n]
        waT[:, c * P:c * P + n] = W_amp[idx].T

    kT = np.zeros((ncp * P, ncp * P), dtype=np.float32)
    for jc in range(ncp):
        nj = CHUNK_REAL[jc]
        jdx = perm[jc, :nj]
        for ic in range(ncp):
            ni = CHUNK_REAL[ic]
            idx = perm[ic, :ni]
            kT[jc * P:jc * P + nj, ic * P:ic * P + ni] = \
                (float(STRIDE) if fast_rot else 1.0) * DT * \
                K[np.ix_(idx, jdx)].T

    nz = [
        (jc, ic)
        for jc in range(ncp)
        for ic in range(ncp)
        if np.any(kT[jc * P:(jc + 1) * P, ic * P:(ic + 1) * P] != 0.0)
    ]
    if (0, 0) not in nz:
        nz.append((0, 0))     # carries the band-sum indicator columns
    for ic in range(1, ncp):
        if not any(i2 == ic for (_, i2) in nz):
            nz.append((ic, ic))
    nz_pairs = tuple(sorted(nz))

    # fuse delta/theta indicator columns into the (0,0) block pads:
    # cols 96,97 tap the sin half (phi_pad=0), 98,99 the cos half (pi/2)
    for cc in (96, 98):
        kT[0:N_DELTA, cc] = 1.0
        kT[N_DELTA:96, cc + 1] = 1.0

    key = (nz_pairs, fast_rot, has_res, ncp)
    if key not in _COMPILED:
        _COMPILED[key] = _build_program(nz_pairs, fast_rot, has_res, ncp)
    nc = _COMPILED[key]

    # const blob: kt blocks + padphi rows (on partitions 96:100)
    KTW = ncp * ncp * P
    PPW = max(BHS)
    constp = np.zeros((P, KTW + PPW), dtype=ml_dtypes.bfloat16)
    for jc in range(ncp):
        for ic in range(ncp):
            constp[:, (jc * ncp + ic) * P:(jc * ncp + ic + 1) * P] = \
                kT[jc * P:(jc + 1) * P, ic * P:(ic + 1) * P]
    constp[98:100, KTW:] = np.float32(PI / 2.0)
    wpp = chunk_pack(wpT).astype(ml_dtypes.bfloat16)
    wap = chunk_pack(waT).astype(ml_dtypes.bfloat16)
    in_maps = []
    for i in range(N_CORES):
        xs = x[i * BL:(i + 1) * BL]
        xsp = chunk_pack(np.ascontiguousarray(xs.T))
        in_maps.append({
            "xbT": xsp.astype(ml_dtypes.bfloat16),
            "wpT": wpp, "waT": wap, "constT": constp, "dtw": dtw,
        })

    res_run = run_bass_kernel_spmd(nc, in_maps, core_ids=list(range(N_CORES)))

    # ---- host-side unshard + exact amp reconstruction (f64) ----
    out = np.empty((BATCH, N_TOTAL), dtype=np.float32)
    NDUMP = (N_STEPS // STRIDE + 1) if fast_rot else N_STEPS
    kk = np.arange(1, N_STEPS + 1)
    dmap = (kk // STRIDE) if fast_rot else (kk - 1)
    ks = kk.astype(np.float64)
    if fast_rot:
        rotd = ks * A_band[0]
        rott = ks * A_band[1]
    else:
        rotd = np.zeros(N_STEPS)
        rott = np.zeros(N_STEPS)

    for i in range(N_CORES):
        r = res_run.results[i]
        amp0v = np.maximum(np.abs(r["amp0"].astype(np.float64)), EPS)
        bsv = r["bsums"].astype(np.float64)      # [4, NDUMP*BL]
        if not fast_rot:
            bsv = -bsv                           # fallback d = -coup sign
        f = np.empty((BL, N_STEPS, 2))
        off = 0
        for h in range(NH):
            bh = BHS[h]
            blk = bsv[:, off:off + NDUMP * bh].reshape(4, NDUMP, bh)
            blk = blk[:, dmap]                    # expand to N_STEPS
            S = blk[0:2]                          # [2(d,t), k, j] sin sums
            C = -blk[2:4]
            R = np.sqrt(S * S + C * C)
            R = np.maximum(R, 1e-30)
            cd = (C[0] * np.cos(rotd)[:, None]
                  - S[0] * np.sin(rotd)[:, None]) / R[0]
            ct = (C[1] * np.cos(rott)[:, None]
                  - S[1] * np.sin(rott)[:, None]) / R[1]
            sl = slice(OFFS[h], OFFS[h] + bh)
            f[sl, :, 0] = 1.0 + DT * PAC * cd.T   # theta-band factor
            f[sl, :, 1] = 1.0 + DT * PAC * ct.T   # gamma-band factor
            off += NDUMP * bh
        Pk = np.cumprod(f, axis=1)
        m = np.minimum.accumulate(Pk, axis=1)
        Pn = Pk[:, -1]
        mn = m[:, -1]
        Pfac = np.ones((BL, 3))
        Efac = np.ones((BL, 3))
        Pfac[:, 1:] = Pn
        Efac[:, 1:] = Pn / mn
        a0 = np.empty((BL, N_TOTAL))
        for c in range(NCH):
            n = CHUNK_REAL[c]
            idx = perm[c, :n]
            a0[:, idx] = amp0v[:n, c * BL:(c + 1) * BL].T
        amp = np.maximum(a0 * Pfac[:, band_of], EPS * Efac[:, band_of])
        out[i * BL:(i + 1) * BL] = amp.astype(np.float32)
    return out
